# revision 1
# baseline (speedup 1.0000x reference)
#!/usr/bin/env python3
"""2-layer GAT on 8 NeuronCores (Bass/Tile).

Sharding: nodes partitioned across 8 cores by dst id (graph parallel).
Per-node features computed locally, per-node gather tables allgathered,
per-edge source rows fetched with dma_gather, segment softmax/aggregation
via indicator matmuls on the tensor engine.
"""
import sys
import numpy as np

sys.path.insert(0, "/opt/pypackages")
sys.path.insert(0, "/opt/trn_rl_repo")

import concourse.bass as bass
import concourse.bacc as bacc
import concourse.tile as tile
import concourse.mybir as mybir
from concourse.bass_utils import run_bass_kernel_spmd

# problem constants
N = 100000
F_IN = 512
NHID = 16
HEADS = 8
NCLASS = 40
E = 1600000
NEG_SLOPE = 0.2

NCORES = 8
NPC = N // NCORES            # 12500 nodes per core
DCH = 128                    # dsts per chunk
NCH = (NPC + DCH - 1) // DCH  # 98 chunks
NPAD = NCH * DCH             # 12544 padded rows per core shard
NSCH = 4
SCHW = (NPAD * NCORES) // NSCH  # 25088 src rows per index window (int16-safe)

ROW1 = 256    # fp16 elems per L1 table row (512B): [h1 128 | asrc1 8 | pad]
ROW2 = 128    # fp16 elems per L2 table row (256B): [h2 40 | one | asrc2 | pad]
ROWA = 128    # fp16 elems per adst-replica row (256B)

F16 = mybir.dt.float16
F32 = mybir.dt.float32
I16 = mybir.dt.int16


def _wrap_block(v):
    """Wrap a 1-D int16 block (len % 16 == 0) into dma_gather idx layout
    [16, L/16], replicated to 128 partitions."""
    w = v.reshape(-1, 16).T
    return np.tile(w, (8, 1))



def _dma_gather_raw(gp, out_ap, in_ap, idxs_ap, num_idxs, elem_size, elem_step,
                    queue_num=0):
    """dma_gather allowing elem_size (bytes read per row) that is not a
    multiple of 256B; the table row stride (elem_step) still must be."""
    from concourse.bass import exact_div
    stride_bytes = elem_step * mybir.dt.size(in_ap.dtype)
    stride_bytes_256 = exact_div(stride_bytes, 256)
    _in_ap = gp.lower_ap_dma(in_ap, for_custom_bir_dma=True)
    _idxs_ap = gp.lower_ap(idxs_ap)
    _out_ap = gp.lower_ap(out_ap)
    return gp.add_instruction(
        mybir.InstDMAGatherAnt(
            name=gp.bass.get_next_instruction_name(),
            ins=[*_in_ap, _idxs_ap, gp.lower_val_access(gp.to_reg(num_idxs))],
            outs=[_out_ap],
            transpose=False, num_idxs=num_idxs, elem_size=elem_size,
            stride_bytes_256=stride_bytes_256, gen_mode=0,
            single_packet=False, queue_num=queue_num,
            sbuf_tokens_per_rank=0, sbuf_free_dim_per_rank=0,
            sbuf_free_dim_pad_per_rank=0, sbuf_byte_offset=0))


def _prep(x, edge_index, W1, att_src1, att_dst1, W2, att_src2, att_dst2,
          b1=None, b2=None):
    """Host-side sharding/packing. Returns (in_maps, shapes) where shapes is
    the static cell structure shared by all cores."""
    src = np.concatenate([edge_index[0], np.arange(N, dtype=np.int64)])
    dst = np.concatenate([edge_index[1], np.arange(N, dtype=np.int64)])

    core = dst // NPC
    dl = (dst - core * NPC).astype(np.int64)      # local dst 0..12499
    dch = dl >> 7                                  # dst chunk 0..97
    s_pad = (src // NPC) * NPAD + (src % NPC)      # padded global src row
    sch = s_pad // SCHW
    sloc = (s_pad - sch * SCHW).astype(np.int64)   # 0..25087 (int16 ok)

    cell = ((core * NCH + dch) * NSCH + sch).astype(np.int64)
    order = np.argsort(cell * (SCHW + 1) + sloc, kind="stable")
    cell_s, sloc_s, dl_s = cell[order], sloc[order], dl[order]

    ncells = NCORES * NCH * NSCH
    counts = np.bincount(cell_s, minlength=ncells).reshape(NCORES, NCH * NSCH)
    shapes = (np.ceil(counts.max(axis=0) / 128.0).astype(np.int64) * 128)  # [NCH*NSCH]
    cell_starts = np.concatenate([[0], np.cumsum(shapes)])                 # per-core stream offsets
    t_total = int(cell_starts[-1]) // 128

    # rank of each edge within its cell
    group_start = np.concatenate([[0], np.cumsum(counts.reshape(-1))])
    first_of_cell = group_start[cell_s]
    rank = np.arange(len(cell_s)) - first_of_cell
    # destination position within the owning core's padded stream
    pos = cell_starts[cell_s % (NCH * NSCH)] + rank
    core_s = cell_s // (NCH * NSCH)

    L = t_total * 128
    idx1 = np.zeros((NCORES, L), dtype=np.int16)
    idxd = np.zeros((NCORES, L), dtype=np.int16)
    dstloc = np.full((NCORES, L), 255.0, dtype=np.float16)
    idx1[core_s, pos] = sloc_s.astype(np.int16)
    idxd[core_s, pos] = dl_s.astype(np.int16)
    dstloc[core_s, pos] = (dl_s & 127).astype(np.float16)

    # per-chunk tile counts and cell layout
    shapes2 = shapes.reshape(NCH, NSCH)
    # wrapped idx streams
    IDX1 = np.zeros((NCORES, 128, L // 16), dtype=np.int16)
    IDXD = np.zeros((NCORES, 128, L // 16), dtype=np.int16)
    for k in range(NCORES):
        off = 0
        for d in range(NCH):
            chunk_len = int(shapes2[d].sum())
            if chunk_len:
                blk = idxd[k, off:off + chunk_len]
                IDXD[k][:, off // 16:(off + chunk_len) // 16] = _wrap_block(blk)
            coff = off
            for s in range(NSCH):
                cl = int(shapes2[d, s])
                if cl:
                    blk = idx1[k, coff:coff + cl]
                    IDX1[k][:, coff // 16:(coff + cl) // 16] = _wrap_block(blk)
                coff += cl
            off += chunk_len
    DSTLOC = dstloc.reshape(NCORES, t_total, 128).transpose(0, 2, 1).copy()

    # weights
    asrc1 = att_src1.reshape(HEADS, NHID)
    adst1 = att_dst1.reshape(HEADS, NHID)
    W1r = W1.reshape(F_IN, HEADS, NHID)
    W1as = np.einsum("khc,hc->kh", W1r, asrc1)     # [512, 8]
    W1ad = np.einsum("khc,hc->kh", W1r, adst1)
    W1ext = np.concatenate([W1, W1as, W1ad], axis=1).astype(np.float16)  # [512, 144]
    W2as = W2 @ att_src2.reshape(NCLASS, 1)        # [128, 1]
    W2ad = W2 @ att_dst2.reshape(NCLASS, 1)
    W2ext = np.concatenate([W2, W2as, W2ad], axis=1).astype(np.float16)  # [128, 42]

    iota = np.broadcast_to(np.arange(128, dtype=np.float16), (128, 128)).copy()

    in_maps = []
    for k in range(NCORES):
        xs = x[k * NPC:(k + 1) * NPC]              # [12500, 512]
        xT = np.zeros((F_IN, NPAD), dtype=np.float16)
        xT[:, :NPC] = xs.T
        in_maps.append({
            "xT": xT,
            "W1ext": W1ext,
            "W2ext": W2ext,
            "IDX1": IDX1[k],
            "IDXD": IDXD[k],
            "DSTLOC": DSTLOC[k],
            "iota": iota,
            "B1": (np.zeros((1, 128), np.float32) if b1 is None
                   else np.asarray(b1, np.float32).reshape(1, 128)),
            "B2": (np.zeros((1, NCLASS), np.float32) if b2 is None
                   else np.asarray(b2, np.float32).reshape(1, NCLASS)),
        })
    return in_maps, shapes2


def _build(shapes2, nch=NCH, phases="ABCDE", clevel=9):
    """Build the Bass module given the static cell structure [NCH, NSCH]."""
    from concourse.masks import make_identity

    t_chunks = [int(shapes2[d].sum()) // 128 for d in range(NCH)]
    t_total = sum(t_chunks)
    TMAX = max(t_chunks)

    nc = bacc.Bacc("TRN2", target_bir_lowering=False, debug=False,
                   enable_asserts=False, num_devices=NCORES,
                   num_swdge_queues=4)

    xT = nc.dram_tensor("xT", [F_IN, NPAD], F16, kind="ExternalInput")
    W1e = nc.dram_tensor("W1ext", [F_IN, 144], F16, kind="ExternalInput")
    W2e = nc.dram_tensor("W2ext", [128, 42], F16, kind="ExternalInput")
    IDX1 = nc.dram_tensor("IDX1", [128, t_total * 8], I16, kind="ExternalInput")
    IDXD = nc.dram_tensor("IDXD", [128, t_total * 8], I16, kind="ExternalInput")
    DSTLOC = nc.dram_tensor("DSTLOC", [128, t_total], F16, kind="ExternalInput")
    IOTA = nc.dram_tensor("iota", [128, 128], F16, kind="ExternalInput")
    B1 = nc.dram_tensor("B1", [1, 128], F32, kind="ExternalInput")
    B2 = nc.dram_tensor("B2", [1, NCLASS], F32, kind="ExternalInput")
    OUT = nc.dram_tensor("out", [NPAD, NCLASS], F32, kind="ExternalOutput")

    tab1_sh = nc.dram_tensor("tab1_sh", [NPAD, ROW1], F16, kind="Internal")
    tab1 = nc.dram_tensor("tab1", [NPAD * NCORES, ROW1], F16, kind="Internal",
                          addr_space="Shared")
    tab2_sh = nc.dram_tensor("tab2_sh", [NPAD, ROW2], F16, kind="Internal")
    tab2 = nc.dram_tensor("tab2", [NPAD * NCORES, ROW2], F16, kind="Internal",
                          addr_space="Shared")
    adr1 = nc.dram_tensor("adr1", [NPAD, ROWA], F16, kind="Internal")
    adr2 = nc.dram_tensor("adr2", [NPAD, ROWA], F16, kind="Internal")

    eq = mybir.AluOpType.is_equal
    mult = mybir.AluOpType.mult
    amax = mybir.AluOpType.max
    aadd = mybir.AluOpType.add
    sub = mybir.AluOpType.subtract
    AF = mybir.ActivationFunctionType
    AX = mybir.AxisListType

    with tile.TileContext(nc) as tc:
        if "A" in phases:
            _phase_a(nc, tc, nch, xT, W1e, tab1_sh, adr1)
        if "B" in phases:
            nc.gpsimd.collective_compute(
                "AllGather", mybir.AluOpType.bypass,
                replica_groups=[list(range(NCORES))],
                ins=[tab1_sh[:]], outs=[tab1[:]])
        if "C" in phases:
            _phase_c(nc, tc, nch, shapes2, t_chunks, TMAX, make_identity,
                     IDX1, IDXD, DSTLOC, IOTA, B1, W2e, tab1, adr1, tab2_sh, adr2,
                     eq, mult, amax, aadd, AF, clevel)
        if "D" in phases:
            nc.gpsimd.collective_compute(
                "AllGather", mybir.AluOpType.bypass,
                replica_groups=[list(range(NCORES))],
                ins=[tab2_sh[:]], outs=[tab2[:]])
        if "E" in phases:
            _phase_e(nc, tc, nch, shapes2, t_chunks, TMAX,
                     IDX1, IDXD, DSTLOC, IOTA, B2, tab2, adr2, OUT,
                     eq, mult, amax, aadd, sub, AF, AX)

    nc.compile()
    return nc


def _phase_a(nc, tc, nch, xT, W1e, tab1_sh, adr1):
    with tc.tile_pool(name="sbA", bufs=1) as sba, \
         tc.tile_pool(name="sbA2", bufs=4) as sba2, \
         tc.tile_pool(name="psA", bufs=4, space="PSUM") as psa:
        xts = [sba.tile([128, NPAD], F16, tag=f"xt{k}", name=f"xt{k}")
               for k in range(4)]
        w1s = [sba.tile([128, 144], F16, tag=f"w1{k}", name=f"w1{k}")
               for k in range(4)]
        for k in range(4):
            nc.sync.dma_start(xts[k][:], xT[k * 128:(k + 1) * 128, :])
            nc.sync.dma_start(w1s[k][:], W1e[k * 128:(k + 1) * 128, :])
        for nt in range(nch):
            ps = psa.tile([128, 144], F32, tag="psA", name="psA")
            for k in range(4):
                nc.tensor.matmul(ps[:], lhsT=xts[k][:, nt * 128:(nt + 1) * 128],
                                 rhs=w1s[k][:], start=(k == 0), stop=(k == 3))
            row = sba2.tile([128, 136], F16, tag="row", name="row")
            nc.vector.tensor_copy(row[:], ps[:, 0:136])
            nc.sync.dma_start(tab1_sh[nt * 128:(nt + 1) * 128, 0:136], row[:])
            t8 = sba2.tile([128, 8], F16, tag="t8", name="t8")
            nc.vector.tensor_copy(t8[:], ps[:, 136:144])
            nc.sync.dma_start(adr1[nt * 128:(nt + 1) * 128, 0:8], t8[:])


def _phase_c(nc, tc, nch, shapes2, t_chunks, TMAX, make_identity,
             IDX1, IDXD, DSTLOC, IOTA, B1, W2e, tab1, adr1, tab2_sh, adr2,
             eq, mult, amax, aadd, AF, clevel=9):
    with tc.tile_pool(name="sbC", bufs=1) as sbc, \
         tc.tile_pool(name="sbC2", bufs=3) as sb2, \
         tc.tile_pool(name="psC", bufs=2, space="PSUM") as psc:
        iot = sbc.tile([128, 128], F16, tag="iota", name="iotc")
        nc.sync.dma_start(iot[:], IOTA[:])
        ident = sbc.tile([128, 128], F16, tag="ident", name="ident")
        make_identity(nc, ident[:])
        w2s = sbc.tile([128, 42], F16, tag="w2s", name="w2s")
        nc.sync.dma_start(w2s[:], W2e[:])
        b1t = sbc.tile([128, 128], F32, tag="b1t", name="b1t")
        nc.sync.dma_start(b1t[:], B1[:].to_broadcast([128, 128]))

        off = 0  # tile offset into the edge stream
        for d in range(nch):
            T = t_chunks[d]
            if T == 0:
                continue
            i1 = sb2.tile([128, TMAX * 8], I16, tag="i1", name="i1")
            nc.sync.dma_start(i1[:, 0:T * 8], IDX1[:, off * 8:(off + T) * 8])
            idd = sb2.tile([128, TMAX * 8], I16, tag="idd", name="idd")
            nc.sync.dma_start(idd[:, 0:T * 8], IDXD[:, off * 8:(off + T) * 8])
            dlc = sb2.tile([128, TMAX], F16, tag="dlc", name="dlc")
            nc.sync.dma_start(dlc[:, 0:T], DSTLOC[:, off:off + T])

            g1 = sb2.tile([128, TMAX * ROW1], F16, tag="g1", name="g1")
            coff = 0
            for s in range(NSCH):
                cl = int(shapes2[d, s])
                if cl == 0:
                    continue
                if clevel >= 1:
                    nc.gpsimd.dma_gather(
                        out_ap=g1[:, coff * 2:(coff * 2 + (cl // 128) * ROW1)]
                        .rearrange("p (t e) -> p t e", e=ROW1),
                        in_ap=tab1[s * SCHW:(s + 1) * SCHW, :],
                        idxs_ap=i1[:, coff // 16:(coff + cl) // 16],
                        num_idxs=cl, num_idxs_reg=cl, elem_size=ROW1, single_packet=False)
                coff += cl
            ga = sb2.tile([128, TMAX * 8], F16, tag="ga", name="ga")
            nedge = T * 128
            _dma_gather_raw(nc.gpsimd,
                            ga[:, 0:T * 8].rearrange("p (t e) -> p t e", e=8),
                            adr1[:], idd[:, 0:nedge // 16], nedge, 8, ROWA,
                            queue_num=d % 4)

            if clevel < 2:
                dbg = sb2.tile([128, 128], F16, tag="dbg", name="dbg")
                nc.vector.tensor_copy(dbg[:], ga[:, 0:128] if clevel < 1 else g1[:, 0:128])
                nc.sync.dma_start(tab2_sh[d * 128:(d + 1) * 128, 0:128], dbg[:])
                off += T
                continue
            g13 = g1[:, 0:T * ROW1].rearrange("p (t e) -> p t e", e=ROW1)
            ga3 = ga[:, 0:T * 8].rearrange("p (t e) -> p t e", e=8)

            ind = sb2.tile([128, TMAX * 128], F16, tag="ind", name="ind")
            ind3 = ind[:, 0:T * 128].rearrange("p (t s) -> p t s", s=128)
            nc.vector.tensor_tensor(
                out=ind3,
                in0=iot[:].rearrange("p (t s) -> p t s", t=1)
                .to_broadcast([128, T, 128]),
                in1=dlc[:, 0:T].rearrange("p (t s) -> p t s", s=1)
                .to_broadcast([128, T, 128]),
                op=eq)

            att = sb2.tile([128, TMAX * 8], F16, tag="att", name="att")
            at3 = att[:, 0:T * 8].rearrange("p (t h) -> p t h", h=8)
            nc.vector.tensor_tensor(out=at3, in0=g13[:, :, 128:136],
                                    in1=ga3[:, :, 0:8], op=aadd)
            nc.vector.scalar_tensor_tensor(
                out=at3, in0=at3, scalar=NEG_SLOPE, in1=at3, op0=mult, op1=amax)
            wst = sb2.tile([128, TMAX * 8], F16, tag="wst", name="wst")
            nc.scalar.activation(out=wst[:, 0:T * 8], in_=att[:, 0:T * 8],
                                 func=AF.Exp)

            if clevel < 3:
                dbg = sb2.tile([128, 128], F16, tag="dbg", name="dbg")
                nc.vector.tensor_copy(dbg[:, 0:120], ind[:, 0:120])
                nc.vector.tensor_copy(dbg[:, 120:128], wst[:, 0:8])
                nc.sync.dma_start(tab2_sh[d * 128:(d + 1) * 128, 0:128], dbg[:])
                off += T
                continue
            ust = sb2.tile([128, TMAX * 136], F16, tag="ust", name="ust")
            us3 = ust[:, 0:T * 136].rearrange("p (t e) -> p t e", e=136)
            w3 = wst[:, 0:T * 8].rearrange("p (t h) -> p t h", h=8)
            nc.vector.tensor_tensor(
                out=ust[:, 0:T * 136].rearrange("p (t e) -> p t e", e=136)[:, :, 0:128]
                .rearrange("p t (h c) -> p t h c", c=NHID),
                in0=g1[:, 0:T * ROW1].rearrange("p (t e) -> p t e", e=ROW1)[:, :, 0:128]
                .rearrange("p t (h c) -> p t h c", c=NHID),
                in1=wst[:, 0:T * 8].rearrange("p (t h c) -> p t h c", h=8, c=1)
                .to_broadcast([128, T, 8, NHID]),
                op=mult)
            nc.vector.tensor_copy(us3[:, :, 128:136], w3)

            ps1 = psc.tile([128, 136], F32, tag="ps1", name="ps1")
            for t in range(T):
                nc.tensor.matmul(ps1[:], lhsT=ind[:, t * 128:(t + 1) * 128],
                                 rhs=ust[:, t * 136:(t + 1) * 136],
                                 start=(t == 0), stop=(t == T - 1))

            if clevel < 4:
                dbg = sb2.tile([128, 128], F16, tag="dbg", name="dbg")
                nc.vector.tensor_copy(dbg[:], ps1[:, 0:128])
                nc.sync.dma_start(tab2_sh[d * 128:(d + 1) * 128, 0:128], dbg[:])
                off += T
                continue
            rc = sb2.tile([128, 8], F32, tag="rc", name="rc")
            nc.vector.reciprocal(rc[:], ps1[:, 128:136])
            o1 = sb2.tile([128, 128], F32, tag="o1", name="o1")
            nc.vector.tensor_tensor(
                out=o1[:].rearrange("p (h c) -> p h c", c=NHID),
                in0=ps1[:, 0:128].rearrange("p (h c) -> p h c", c=NHID),
                in1=rc[:].rearrange("p (h c) -> p h c", c=1)
                .to_broadcast([128, 8, NHID]),
                op=mult)
            nc.vector.tensor_tensor(out=o1[:], in0=o1[:], in1=b1t[:], op=aadd)
            # elu = max(x,0) + (exp(min(x,0)) - 1)
            t1 = sb2.tile([128, 128], F32, tag="t1", name="t1")
            nc.vector.tensor_scalar_min(t1[:], o1[:], 0.0)
            t2 = sb2.tile([128, 128], F32, tag="t2", name="t2")
            nc.scalar.activation(out=t2[:], in_=t1[:], func=AF.Exp)
            nc.vector.tensor_scalar_add(t2[:], t2[:], -1.0)
            nc.vector.tensor_scalar_max(t1[:], o1[:], 0.0)
            elu = sb2.tile([128, 128], F16, tag="elu", name="elu")
            nc.vector.tensor_tensor(out=elu[:], in0=t1[:], in1=t2[:], op=aadd)

            if clevel < 5:
                nc.sync.dma_start(tab2_sh[d * 128:(d + 1) * 128, 0:128], elu[:])
                off += T
                continue
            psT = psc.tile([128, 128], F16, tag="psT", name="psT")
            nc.tensor.transpose(psT[:], elu[:], ident[:])
            eluT = sb2.tile([128, 128], F16, tag="eluT", name="eluT")
            nc.vector.tensor_copy(eluT[:], psT[:])
            ps2a = psc.tile([128, 42], F32, tag="ps2a", name="ps2a")
            nc.tensor.matmul(ps2a[:], lhsT=eluT[:], rhs=w2s[:],
                             start=True, stop=True)

            h2r = sb2.tile([128, ROW2], F16, tag="h2r", name="h2r")
            nc.vector.tensor_copy(h2r[:, 0:NCLASS], ps2a[:, 0:NCLASS])
            nc.vector.memset(h2r[:, NCLASS:NCLASS + 1], 1.0)
            nc.vector.tensor_copy(h2r[:, NCLASS + 1:NCLASS + 2],
                                  ps2a[:, NCLASS:NCLASS + 1])
            nc.sync.dma_start(tab2_sh[d * 128:(d + 1) * 128, 0:NCLASS + 2],
                              h2r[:, 0:NCLASS + 2])
            a2c = sb2.tile([128, 8], F16, tag="a2c", name="a2c")
            nc.vector.tensor_copy(
                a2c[:].rearrange("p (r h) -> p r h", h=1),
                ps2a[:, 41:42].rearrange("p (r h) -> p r h", r=1)
                .to_broadcast([128, 8, 1]))
            nc.sync.dma_start(adr2[d * 128:(d + 1) * 128, 0:8], a2c[:])
            off += T


def _phase_e(nc, tc, nch, shapes2, t_chunks, TMAX,
             IDX1, IDXD, DSTLOC, IOTA, B2, tab2, adr2, OUT,
             eq, mult, amax, aadd, sub, AF, AX):
    with tc.tile_pool(name="sbE", bufs=1) as sbe, \
         tc.tile_pool(name="sbE2", bufs=3) as se2, \
         tc.tile_pool(name="psE", bufs=4, space="PSUM") as pse:
        iot = sbe.tile([128, 128], F16, tag="iotaE", name="iote")
        nc.sync.dma_start(iot[:], IOTA[:])
        b2t = sbe.tile([128, NCLASS], F32, tag="b2t", name="b2t")
        nc.sync.dma_start(b2t[:], B2[:].to_broadcast([128, NCLASS]))
        off = 0
        for d in range(nch):
            T = t_chunks[d]
            if T == 0:
                continue
            i1 = se2.tile([128, TMAX * 8], I16, tag="i1e", name="i1e")
            nc.sync.dma_start(i1[:, 0:T * 8], IDX1[:, off * 8:(off + T) * 8])
            idd = se2.tile([128, TMAX * 8], I16, tag="idde", name="idde")
            nc.sync.dma_start(idd[:, 0:T * 8], IDXD[:, off * 8:(off + T) * 8])
            dlc = se2.tile([128, TMAX], F16, tag="dlce", name="dlce")
            nc.sync.dma_start(dlc[:, 0:T], DSTLOC[:, off:off + T])

            g2 = se2.tile([128, TMAX * 42], F16, tag="g2", name="g2")
            coff = 0
            for s in range(NSCH):
                cl = int(shapes2[d, s])
                if cl == 0:
                    continue
                _dma_gather_raw(nc.gpsimd,
                                g2[:, (coff // 128) * 42:((coff + cl) // 128) * 42]
                                .rearrange("p (t e) -> p t e", e=42),
                                tab2[s * SCHW:(s + 1) * SCHW, :],
                                i1[:, coff // 16:(coff + cl) // 16], cl, 42, ROW2,
                                queue_num=s)
                coff += cl
            ga2 = se2.tile([128, TMAX * 8], F16, tag="ga2", name="ga2")
            nedge = T * 128
            _dma_gather_raw(nc.gpsimd,
                            ga2[:, 0:T * 8].rearrange("p (t e) -> p t e", e=8),
                            adr2[:], idd[:, 0:nedge // 16], nedge, 8, ROWA,
                            queue_num=d % 4)

            g23 = g2[:, 0:T * 42].rearrange("p (t e) -> p t e", e=42)
            ga23 = ga2[:, 0:T * 8].rearrange("p (t e) -> p t e", e=8)

            ind = se2.tile([128, TMAX * 128], F16, tag="inde", name="inde")
            ind3 = ind[:, 0:T * 128].rearrange("p (t s) -> p t s", s=128)
            nc.vector.tensor_tensor(
                out=ind3,
                in0=iot[:].rearrange("p (t s) -> p t s", t=1)
                .to_broadcast([128, T, 128]),
                in1=dlc[:, 0:T].rearrange("p (t s) -> p t s", s=1)
                .to_broadcast([128, T, 128]),
                op=eq)

            at2 = se2.tile([128, TMAX], F16, tag="at2", name="at2")
            at23 = at2[:, 0:T].rearrange("p (t h) -> p t h", h=1)
            nc.vector.tensor_tensor(out=at23,
                                    in0=g23[:, :, NCLASS + 1:NCLASS + 2],
                                    in1=ga23[:, :, 0:1], op=aadd)
            nc.vector.scalar_tensor_tensor(
                out=at23, in0=at23, scalar=NEG_SLOPE, in1=at23,
                op0=mult, op1=amax)
            w2t = se2.tile([128, TMAX], F16, tag="w2t", name="w2t")
            nc.scalar.activation(out=w2t[:, 0:T], in_=at2[:, 0:T], func=AF.Exp)

            gw = se2.tile([128, TMAX * 42], F16, tag="gw", name="gw")
            nc.vector.tensor_tensor(
                out=gw[:, 0:T * 42].rearrange("p (t e) -> p t e", e=42),
                in0=g23,
                in1=w2t[:, 0:T].rearrange("p (t s) -> p t s", s=1)
                .to_broadcast([128, T, 42]),
                op=mult)

            ps2 = pse.tile([128, NCLASS + 1], F32, tag="ps2", name="ps2")
            for t in range(T):
                nc.tensor.matmul(ps2[:], lhsT=ind[:, t * 128:(t + 1) * 128],
                                 rhs=gw[:, t * 42:t * 42 + NCLASS + 1],
                                 start=(t == 0), stop=(t == T - 1))

            rc2 = se2.tile([128, 1], F32, tag="rc2", name="rc2")
            nc.vector.reciprocal(rc2[:], ps2[:, NCLASS:NCLASS + 1])
            lg = se2.tile([128, NCLASS], F32, tag="lg", name="lg")
            nc.vector.scalar_tensor_tensor(out=lg[:], in0=ps2[:, 0:NCLASS],
                                           scalar=rc2[:], in1=b2t[:],
                                           op0=mult, op1=aadd)
            ex = se2.tile([128, NCLASS], F32, tag="ex", name="ex")
            sm = se2.tile([128, 1], F32, tag="sm", name="sm")
            nc.scalar.activation(out=ex[:], in_=lg[:], func=AF.Exp,
                                 accum_out=sm[:])
            ls = se2.tile([128, 1], F32, tag="ls", name="ls")
            nc.scalar.activation(out=ls[:], in_=sm[:], func=AF.Ln)
            fin = se2.tile([128, NCLASS], F32, tag="fin", name="fin")
            nc.vector.tensor_scalar(out=fin[:], in0=lg[:], scalar1=ls[:],
                                    scalar2=None, op0=sub)
            nc.sync.dma_start(OUT[d * 128:(d + 1) * 128, :], fin[:])
            off += T


_CACHE = {}


def kernel(x, edge_index, W1, att_src1, att_dst1, b1, W2, att_src2, att_dst2, b2):
    x = np.asarray(x, dtype=np.float32)
    edge_index = np.asarray(edge_index)
    in_maps, shapes2 = _prep(np.asarray(x), edge_index,
                             np.asarray(W1), np.asarray(att_src1),
                             np.asarray(att_dst1), np.asarray(W2),
                             np.asarray(att_src2), np.asarray(att_dst2),
                             b1=b1, b2=b2)
    key = shapes2.tobytes()
    if key not in _CACHE:
        _CACHE[key] = _build(shapes2)
    nc = _CACHE[key]
    res = run_bass_kernel_spmd(nc, in_maps, core_ids=list(range(NCORES)))
    out = np.concatenate([res.results[k]["out"][:NPC] for k in range(NCORES)], axis=0)
    return out.astype(np.float32)



# revision 21
# speedup vs baseline: 13.1770x; 13.1770x over previous
#!/usr/bin/env python3
"""2-layer GAT on 8 NeuronCores (Bass/Tile).

Sharding: nodes partitioned across 8 cores by dst id (graph parallel).
Layer tables (node features + attention scalars) are computed locally and
allgathered; per-edge source rows are fetched with dma_gather; per-edge
dst-attention values are computed with one-hot-transpose matmuls on the
tensor engine (no per-edge DMA gather); segment softmax/aggregation via
indicator matmuls.

Edge streams are grouped as (chunk-group g, src-window s) blocks so each
dma_gather instruction covers a whole block (~G chunks' edges for one
source window), minimizing GPSIMD descriptor-generation launches.
"""
import sys
import numpy as np

sys.path.insert(0, "/opt/pypackages")
sys.path.insert(0, "/opt/trn_rl_repo")

import concourse.bass as bass
import concourse.bacc as bacc
import concourse.tile as tile
import concourse.mybir as mybir
from concourse.bass_utils import run_bass_kernel_spmd

# problem constants
N = 100000
F_IN = 512
NHID = 16
HEADS = 8
NCLASS = 40
E = 1600000
NEG_SLOPE = 0.2

NCORES = 8
NPC = N // NCORES             # 12500 nodes per core
DCH = 128                     # dsts per chunk
NCH = (NPC + DCH - 1) // DCH  # 98 chunks
NPAD = NCH * DCH              # 12544 padded rows per core shard
NSCH = 4
SCHW = (NPAD * NCORES) // NSCH  # 25088 src rows per index window (int16-safe)
G = 5                         # chunks per group
NG = (NCH + G - 1) // G       # 20 groups

ROW1 = 256    # fp16 elems per L1 table row (512B): [h1 128 | asrc1 8 | pad]
ROW2 = 128    # fp16 elems per L2 table row (256B): [h2 40 | one | asrc2 | pad]

F16 = mybir.dt.float16
F32 = mybir.dt.float32
I16 = mybir.dt.int16
U8 = mybir.dt.uint8


def _wrap_block(v):
    """Wrap a 1-D int16 block (len % 16 == 0) into dma_gather idx layout
    [16, L/16], replicated to 128 partitions."""
    w = v.reshape(-1, 16).T
    return np.tile(w, (8, 1))


def _dma_gather_raw(gp, out_ap, in_ap, idxs_ap, num_idxs, elem_size, elem_step,
                    queue_num=0, single_packet=False):
    """dma_gather allowing elem_size (bytes read per row) that is not a
    multiple of 256B; the table row stride (elem_step) still must be."""
    from concourse.bass import exact_div
    stride_bytes = elem_step * mybir.dt.size(in_ap.dtype)
    stride_bytes_256 = exact_div(stride_bytes, 256)
    _in_ap = gp.lower_ap_dma(in_ap, for_custom_bir_dma=True)
    _idxs_ap = gp.lower_ap(idxs_ap)
    _out_ap = gp.lower_ap(out_ap)
    return gp.add_instruction(
        mybir.InstDMAGatherAnt(
            name=gp.bass.get_next_instruction_name(),
            ins=[*_in_ap, _idxs_ap, gp.lower_val_access(gp.to_reg(num_idxs))],
            outs=[_out_ap],
            transpose=False, num_idxs=num_idxs, elem_size=elem_size,
            stride_bytes_256=stride_bytes_256, gen_mode=0,
            single_packet=single_packet, queue_num=queue_num,
            sbuf_tokens_per_rank=0, sbuf_free_dim_per_rank=0,
            sbuf_free_dim_pad_per_rank=0, sbuf_byte_offset=0))


class _Layout:
    """Static (core-independent) edge-stream layout shared by all cores."""

    def __init__(self, cl16):
        # cl16: [NCH, NSCH] 16-aligned common cell lengths (block-padded)
        self.cl16 = cl16
        self.groups = []  # per g: dict with per-block and schedule info
        tbase = 0   # global tile base
        ppbase = 0  # global pass-tile base
        for g in range(NG):
            ds = list(range(g * G, min((g + 1) * G, NCH)))
            blocks = []
            for s in range(NSCH):
                cello = {}
                off = 0
                for d in ds:
                    cello[d] = off
                    off += int(cl16[d, s])
                L = off
                assert L % 128 == 0
                T = L // 128
                # passes: (di, pt_lo, pt_hi, lo, hi)
                passes = []
                for i, d in enumerate(ds):
                    cl = int(cl16[d, s])
                    if cl == 0:
                        continue
                    lo, hi = cello[d], cello[d] + cl
                    passes.append((i, lo // 128, (hi + 127) // 128, lo, hi))
                TP = sum(p[2] - p[1] for p in passes)
                blocks.append(dict(L=L, T=T, tbase=tbase, ppbase=ppbase,
                                   passes=passes, cello=cello))
                tbase += T
                ppbase += TP
            self.groups.append(dict(ds=ds, blocks=blocks))
        self.t_total = tbase
        self.tp_total = ppbase

    def key(self):
        return self.cl16.tobytes()


def _make_layout(cnt):
    """cnt: [NCORES, NCH, NSCH] per-cell edge counts -> _Layout."""
    cmax = cnt.max(axis=0)                                   # [NCH, NSCH]
    cl16 = ((cmax + 15) // 16 * 16).astype(np.int64)
    # round each (g, s) block up to a 128 multiple by padding its last
    # nonempty cell (ensures gathers cover whole tiles, no stale lanes)
    for g in range(NG):
        ds = list(range(g * G, min((g + 1) * G, NCH)))
        for s in range(NSCH):
            L = int(cl16[ds, s].sum())
            pad = (-L) % 128
            if pad:
                nz = [d for d in ds if cl16[d, s] > 0]
                tgt = nz[-1] if nz else ds[-1]
                cl16[tgt, s] += pad
    return _Layout(cl16)


def _prep(x, edge_index, W1, att_src1, att_dst1, W2, att_src2, att_dst2,
          b1=None, b2=None):
    """Host-side sharding/packing. Returns (in_maps, layout)."""
    # self-loops are handled by a dedicated local path (no gather, identity
    # scatter); only the real edges go through the gather streams
    src = np.asarray(edge_index[0], np.int64)
    dst = np.asarray(edge_index[1], np.int64)

    core = dst // NPC
    dl = (dst - core * NPC).astype(np.int64)       # local dst 0..12499
    d = dl >> 7                                    # dst chunk 0..97
    dlc = (dl & 127).astype(np.int64)              # slot within chunk
    s_pad = (src // NPC) * NPAD + (src % NPC)      # padded global src row
    sch = s_pad // SCHW
    sloc = (s_pad - sch * SCHW).astype(np.int64)   # 0..25087 (int16 ok)

    cell = ((core * NCH + d) * NSCH + sch).astype(np.int64)
    cnt = np.bincount(cell, minlength=NCORES * NCH * NSCH) \
            .reshape(NCORES, NCH, NSCH)
    layout = _make_layout(cnt)
    cl16 = layout.cl16

    # dense position of each edge: order within its cell by sloc
    order = np.argsort(cell * (SCHW + 1) + sloc, kind="stable")
    cell_s, sloc_s, dlc_s = cell[order], sloc[order], dlc[order]
    group_start = np.concatenate(
        [[0], np.cumsum(np.bincount(cell_s, minlength=NCORES * NCH * NSCH))])
    rank = np.arange(len(cell_s)) - group_start[cell_s]

    # cell start positions within the per-core stream
    cellstart = np.zeros((NCH, NSCH), np.int64)
    for g in range(NG):
        for s in range(NSCH):
            blk = layout.groups[g]["blocks"][s]
            for dd, off in blk["cello"].items():
                cellstart[dd, s] = blk["tbase"] * 128 + off
    cs_flat = cellstart.reshape(-1)  # [NCH*NSCH]
    cell_mod = cell_s % (NCH * NSCH)
    pos = cs_flat[cell_mod] + rank
    core_s = cell_s // (NCH * NSCH)

    t_total = layout.t_total
    Ltot = t_total * 128
    idx1 = np.zeros((NCORES, Ltot), dtype=np.int16)
    dlcD = np.full((NCORES, Ltot), 255, dtype=np.int16)
    idx1[core_s, pos] = sloc_s.astype(np.int16)
    dlcD[core_s, pos] = dlc_s.astype(np.int16)

    # wrapped idx stream (16-lane wrap, whole stream at once: every block is
    # 128-aligned so 16-alignment holds everywhere)
    IDX1 = np.zeros((NCORES, 128, Ltot // 16), dtype=np.int16)
    for k in range(NCORES):
        IDX1[k] = _wrap_block(idx1[k])

    # pass streams: DLP [128, TP] lane-major, DLPT [1, TP*128] flat
    TP = layout.tp_total
    DLP = np.full((NCORES, 128, TP), 255, dtype=np.uint8)
    DLPT = np.full((NCORES, 1, TP * 128), 255, dtype=np.uint8)
    for g in range(NG):
        for s in range(NSCH):
            blk = layout.groups[g]["blocks"][s]
            eb = blk["tbase"] * 128
            pp = blk["ppbase"]
            for (di, pt_lo, pt_hi, lo, hi) in blk["passes"]:
                for pt in range(pt_lo, pt_hi):
                    base = pt * 128
                    lanes = np.arange(base, base + 128)
                    vals = np.where(
                        (lanes >= lo) & (lanes < hi),
                        dlcD[:, eb + base:eb + base + 128], 255).astype(np.uint8)
                    DLP[:, :, pp] = vals
                    DLPT[:, 0, pp * 128:(pp + 1) * 128] = vals
                    pp += 1

    # weights
    asrc1 = att_src1.reshape(HEADS, NHID)
    adst1 = att_dst1.reshape(HEADS, NHID)
    W1r = W1.reshape(F_IN, HEADS, NHID)
    W1as = np.einsum("khc,hc->kh", W1r, asrc1)     # [512, 8]
    W1ad = np.einsum("khc,hc->kh", W1r, adst1)
    W1ext = np.concatenate([W1, W1as, W1ad], axis=1).astype(np.float16)  # [512, 144]
    W2as = W2 @ att_src2.reshape(NCLASS, 1)        # [128, 1]
    W2ad = W2 @ att_dst2.reshape(NCLASS, 1)
    W2ext = np.concatenate([W2, W2as, W2ad], axis=1).astype(np.float16)  # [128, 42]

    iota = np.broadcast_to(np.arange(128, dtype=np.uint8), (128, 128)).copy()
    iotc = np.broadcast_to(np.arange(128, dtype=np.uint8)[:, None],
                           (128, 128)).copy()

    in_maps = []
    for k in range(NCORES):
        xs = x[k * NPC:(k + 1) * NPC]              # [12500, 512]
        xT = np.zeros((F_IN, NPAD), dtype=np.float16)
        xT[:, :NPC] = xs.T
        in_maps.append({
            "xT": xT,
            "W1ext": W1ext,
            "W2ext": W2ext,
            "IDX1": IDX1[k],
            "DLP": DLP[k],
            "DLPT": DLPT[k],
            "iota": iota,
            "iotc": iotc,
            "B1": (np.zeros((1, 128), np.float32) if b1 is None
                   else np.asarray(b1, np.float32).reshape(1, 128)),
            "B2": (np.zeros((1, NCLASS), np.float32) if b2 is None
                   else np.asarray(b2, np.float32).reshape(1, NCLASS)),
        })
    return in_maps, layout


def _build(layout, phases="ABCDE", clevel=9, bufs3=4, bufsp=3,
           sp=False, split_coll=False):
    """Build the Bass module given the static layout."""
    from concourse.masks import make_identity

    t_total = layout.t_total
    tp_total = layout.tp_total
    TMAX = max(b["T"] for gr in layout.groups for b in gr["blocks"])
    TPMAX = max(sum(p[2] - p[1] for p in b["passes"])
                for gr in layout.groups for b in gr["blocks"])
    TPGMAX = max(sum(sum(p[2] - p[1] for p in b["passes"])
                     for b in gr["blocks"]) for gr in layout.groups)
    L16GMAX = max(sum(b["L"] for b in gr["blocks"]) // 16
                  for gr in layout.groups)

    nc = bacc.Bacc("TRN2", target_bir_lowering=False, debug=False,
                   enable_asserts=False, num_devices=NCORES,
                   num_swdge_queues=4)

    xT = nc.dram_tensor("xT", [F_IN, NPAD], F16, kind="ExternalInput")
    W1e = nc.dram_tensor("W1ext", [F_IN, 144], F16, kind="ExternalInput")
    W2e = nc.dram_tensor("W2ext", [128, 42], F16, kind="ExternalInput")
    IDX1 = nc.dram_tensor("IDX1", [128, t_total * 8], I16, kind="ExternalInput")
    DLP = nc.dram_tensor("DLP", [128, tp_total], U8, kind="ExternalInput")
    DLPT = nc.dram_tensor("DLPT", [1, tp_total * 128], U8,
                          kind="ExternalInput")
    IOTA = nc.dram_tensor("iota", [128, 128], U8, kind="ExternalInput")
    IOTC = nc.dram_tensor("iotc", [128, 128], U8, kind="ExternalInput")
    B1 = nc.dram_tensor("B1", [1, 128], F32, kind="ExternalInput")
    B2 = nc.dram_tensor("B2", [1, NCLASS], F32, kind="ExternalInput")
    OUT = nc.dram_tensor("out", [NPAD, NCLASS], F32, kind="ExternalOutput")

    tab1_sh = nc.dram_tensor("tab1_sh", [NPAD, ROW1], F16, kind="Internal")
    tab1 = nc.dram_tensor("tab1", [NPAD * NCORES, ROW1], F16, kind="Internal",
                          addr_space="Shared")
    tab2_sh = nc.dram_tensor("tab2_sh", [NPAD, ROW2], F16, kind="Internal")
    tab2 = nc.dram_tensor("tab2", [NPAD * NCORES, ROW2], F16, kind="Internal",
                          addr_space="Shared")
    adr1 = nc.dram_tensor("adr1", [NPAD, 8], F16, kind="Internal")
    adr2 = nc.dram_tensor("adr2", [NPAD, 1], F16, kind="Internal")

    with tile.TileContext(nc) as tc:
        if "A" in phases:
            _phase_a(nc, tc, xT, W1e, tab1_sh, adr1)
        if "B" in phases:
            if split_coll:
                H = NPAD // 2
                tab1v = tab1[:].rearrange("(c r) e -> c r e", c=NCORES)
                nc.gpsimd.collective_compute(
                    "AllGather", mybir.AluOpType.bypass,
                    replica_groups=[list(range(NCORES))],
                    ins=[tab1_sh[0:H, :]], outs=[tab1v[:, 0:H, :]])
                nc.gpsimd.collective_compute(
                    "AllGather", mybir.AluOpType.bypass,
                    replica_groups=[list(range(NCORES))],
                    ins=[tab1_sh[H:NPAD, :]], outs=[tab1v[:, H:NPAD, :]])
            else:
                nc.gpsimd.collective_compute(
                    "AllGather", mybir.AluOpType.bypass,
                    replica_groups=[list(range(NCORES))],
                    ins=[tab1_sh[:]], outs=[tab1[:]])
        if "C" in phases:
            _phase_c(nc, tc, layout, TMAX, TPMAX, TPGMAX, L16GMAX,
                     make_identity,
                     IDX1, DLP, DLPT, IOTA, IOTC, B1, W2e, tab1, tab1_sh,
                     adr1, tab2_sh, adr2, bufs3=bufs3, bufsp=bufsp, sp=sp)
        if "D" in phases:
            if split_coll:
                H = NPAD // 2
                tab2v = tab2[:].rearrange("(c r) e -> c r e", c=NCORES)
                nc.gpsimd.collective_compute(
                    "AllGather", mybir.AluOpType.bypass,
                    replica_groups=[list(range(NCORES))],
                    ins=[tab2_sh[0:H, :]], outs=[tab2v[:, 0:H, :]])
                nc.gpsimd.collective_compute(
                    "AllGather", mybir.AluOpType.bypass,
                    replica_groups=[list(range(NCORES))],
                    ins=[tab2_sh[H:NPAD, :]], outs=[tab2v[:, H:NPAD, :]])
            else:
                nc.gpsimd.collective_compute(
                    "AllGather", mybir.AluOpType.bypass,
                    replica_groups=[list(range(NCORES))],
                    ins=[tab2_sh[:]], outs=[tab2[:]])
        if "E" in phases:
            _phase_e(nc, tc, layout, TMAX, TPMAX, TPGMAX, L16GMAX,
                     IDX1, DLP, DLPT, IOTA, IOTC, B2, tab2, tab2_sh, adr2,
                     OUT, clevel=clevel, bufs3=bufs3, bufsp=bufsp, sp=sp)

    nc.compile()
    return nc


def _phase_a(nc, tc, xT, W1e, tab1_sh, adr1):
    with tc.tile_pool(name="sbA", bufs=1) as sba, \
         tc.tile_pool(name="sbA2", bufs=4) as sba2, \
         tc.tile_pool(name="psA", bufs=4, space="PSUM") as psa:
        xts = [sba.tile([128, NPAD], F16, tag=f"xt{k}", name=f"xt{k}")
               for k in range(4)]
        w1s = [sba.tile([128, 144], F16, tag=f"w1{k}", name=f"w1{k}")
               for k in range(4)]
        for k in range(4):
            nc.sync.dma_start(xts[k][:], xT[k * 128:(k + 1) * 128, :])
            nc.sync.dma_start(w1s[k][:], W1e[k * 128:(k + 1) * 128, :])
        for nt in range(NCH):
            ps = psa.tile([128, 144], F32, tag="psA", name="psA")
            for k in range(4):
                nc.tensor.matmul(ps[:], lhsT=xts[k][:, nt * 128:(nt + 1) * 128],
                                 rhs=w1s[k][:], start=(k == 0), stop=(k == 3))
            row = sba2.tile([128, 136], F16, tag="row", name="row")
            nc.vector.tensor_copy(row[:], ps[:, 0:136])
            nc.sync.dma_start(tab1_sh[nt * 128:(nt + 1) * 128, 0:136], row[:])
            t8 = sba2.tile([128, 8], F16, tag="t8", name="t8")
            nc.vector.tensor_copy(t8[:], ps[:, 136:144])
            nc.sync.dma_start(adr1[nt * 128:(nt + 1) * 128, :], t8[:])


def _emit_block_common(nc, pools, blk, i1g, dlpg, dlptp, iot, ioc,
                       gather_fn, elem):
    """Per-(g,s)-block: gather + ind/indT builds. Returns dict of tiles."""
    sb3 = pools["sb3"]
    T, TPb = blk["T"], sum(p[2] - p[1] for p in blk["passes"])
    if T == 0:
        return None
    gt = gather_fn(blk)

    eq = mybir.AluOpType.is_equal
    ind = sb3.tile([128, pools["TPMAX"] * 128], F16, tag="ind", name="ind")
    nc.vector.tensor_tensor(
        out=ind[:, 0:TPb * 128].rearrange("p (t s) -> p t s", s=128),
        in0=iot[:].rearrange("p (t s) -> p t s", t=1)
        .to_broadcast([128, TPb, 128]),
        in1=dlpg[:, blk["ppg"]:blk["ppg"] + TPb]
        .rearrange("p (t s) -> p t s", s=1).to_broadcast([128, TPb, 128]),
        op=eq)
    indT = sb3.tile([128, pools["TPMAX"] * 128], F16, tag="indT", name="indT")
    nc.vector.tensor_tensor(
        out=indT[:, 0:TPb * 128].rearrange("p (t s) -> p t s", s=128),
        in0=ioc[:].rearrange("p (t s) -> p t s", t=1)
        .to_broadcast([128, TPb, 128]),
        in1=dlptp[:, 0:TPb * 128].rearrange("p (t s) -> p t s", s=128),
        op=eq)
    return dict(g=gt, ind=ind, indT=indT, T=T, TPb=TPb)


def _phase_c(nc, tc, layout, TMAX, TPMAX, TPGMAX, L16GMAX, make_identity,
             IDX1, DLP, DLPT, IOTA, IOTC, B1, W2e, tab1, tab1_sh, adr1,
             tab2_sh, adr2, bufs3=3, bufsp=2, sp=False):
    eq = mybir.AluOpType.is_equal
    mult = mybir.AluOpType.mult
    amax = mybir.AluOpType.max
    aadd = mybir.AluOpType.add
    AF = mybir.ActivationFunctionType

    with tc.tile_pool(name="sbC", bufs=1) as sbc, \
         tc.tile_pool(name="sbC2", bufs=3) as sb2, \
         tc.tile_pool(name="sbC3", bufs=bufs3) as sb3, \
         tc.tile_pool(name="sbCp", bufs=bufsp) as sbp, \
         tc.tile_pool(name="psC1", bufs=1, space="PSUM") as ps1p, \
         tc.tile_pool(name="psCad", bufs=1, space="PSUM") as psadp, \
         tc.tile_pool(name="psCt", bufs=1, space="PSUM") as pstp, \
         tc.tile_pool(name="psC2", bufs=1, space="PSUM") as ps2p:
        iot = sbc.tile([128, 128], U8, tag="iota", name="iotc_")
        nc.sync.dma_start(iot[:], IOTA[:])
        ioc = sbc.tile([128, 128], U8, tag="iotc", name="iocc")
        nc.sync.dma_start(ioc[:], IOTC[:])
        ident = sbc.tile([128, 128], F16, tag="ident", name="ident")
        make_identity(nc, ident[:])
        w2s = sbc.tile([128, 42], F16, tag="w2s", name="w2s")
        nc.sync.dma_start(w2s[:], W2e[:])
        b1t = sbc.tile([128, 128], F32, tag="b1t", name="b1t")
        nc.sync.dma_start(b1t[:], B1[:].to_broadcast([128, 128]))

        pools = dict(sb3=sb3, TPMAX=TPMAX)
        post_prev = [None]

        def gather_fn(blk):
            g1 = sb3.tile([128, TMAX * 136], F16, tag="g1", name="g1")
            _dma_gather_raw(
                nc.gpsimd,
                g1[:, 0:blk["T"] * 136].rearrange("p (t e) -> p t e", e=136),
                tab1[blk["s"] * SCHW:(blk["s"] + 1) * SCHW, :],
                blk["i1g"][:, blk["b16"]:blk["b16"] + blk["L"] // 16],
                blk["L"], 136, ROW1, queue_num=blk["s"],
                single_packet=sp)
            return g1

        for g in range(NG):
            gr = layout.groups[g]
            ds = gr["ds"]
            nd = len(ds)
            blocks = gr["blocks"]
            g16 = sum(b["L"] for b in blocks) // 16
            b16_0 = blocks[0]["tbase"] * 8  # = tbase*128/16
            pp0 = blocks[0]["ppbase"]
            tpg = sum(sum(p[2] - p[1] for p in b["passes"]) for b in blocks)

            i1g = sb2.tile([128, L16GMAX], I16, tag="i1g", name="i1g")
            nc.sync.dma_start(i1g[:, 0:g16], IDX1[:, b16_0:b16_0 + g16])
            dlpg = sb2.tile([128, TPGMAX], U8, tag="dlpg", name="dlpg")
            nc.sync.dma_start(dlpg[:, 0:tpg], DLP[:, pp0:pp0 + tpg])
            adrg = sb2.tile([128, G * 8], F16, tag="adrg", name="adrg")
            nc.sync.dma_start(
                adrg[:, 0:nd * 8].rearrange("p (t h) -> p t h", h=8),
                adr1[ds[0] * 128:(ds[-1] + 1) * 128, :]
                .rearrange("(t p) h -> p t h", p=128))

            # chunk first/last (pass, tile) over the whole group
            first_mm = {}
            last_mm = {}
            for s in range(NSCH):
                for (di, pt_lo, pt_hi, lo, hi) in blocks[s]["passes"]:
                    for pt in range(pt_lo, pt_hi):
                        if di not in first_mm:
                            first_mm[di] = (s, pt, lo)
                        last_mm[di] = (s, pt, lo)

            ps1s = {di: ps1p.tile([128, 136], F32, tag=f"ps1_{di}",
                                  name=f"ps1_{di}")
                    for di in range(nd)}

            # ---- self-loop path: local rows, identity scatter ----
            slr = sb2.tile([128, G * 136], F16, tag="slr", name="slr")
            nc.sync.dma_start(
                slr[:, 0:nd * 136].rearrange("p (t e) -> p t e", e=136),
                tab1_sh[ds[0] * 128:(ds[-1] + 1) * 128, 0:136]
                .rearrange("(t p) e -> p t e", p=128))
            slr3 = slr[:, 0:nd * 136].rearrange("p (t e) -> p t e", e=136)
            sat = sb2.tile([128, G * 8], F16, tag="sat", name="sat")
            sat3 = sat[:, 0:nd * 8].rearrange("p (t h) -> p t h", h=8)
            nc.vector.tensor_tensor(
                out=sat3, in0=slr3[:, :, 128:136],
                in1=adrg[:, 0:nd * 8].rearrange("p (t h) -> p t h", h=8),
                op=aadd)
            nc.vector.scalar_tensor_tensor(
                out=sat3, in0=sat3, scalar=NEG_SLOPE, in1=sat3,
                op0=mult, op1=amax)
            swst = sb2.tile([128, G * 8], F16, tag="swst", name="swst")
            nc.scalar.activation(out=swst[:, 0:nd * 8], in_=sat[:, 0:nd * 8],
                                 func=AF.Exp)
            sust = sb2.tile([128, G * 136], F16, tag="sust", name="sust")
            su3 = sust[:, 0:nd * 136].rearrange("p (t e) -> p t e", e=136)
            nc.vector.tensor_tensor(
                out=su3[:, :, 0:128].rearrange("p t (h c) -> p t h c", c=NHID),
                in0=slr3[:, :, 0:128].rearrange("p t (h c) -> p t h c", c=NHID),
                in1=swst[:, 0:nd * 8]
                .rearrange("p (t h c) -> p t h c", h=8, c=1)
                .to_broadcast([128, nd, 8, NHID]),
                op=mult)
            nc.vector.tensor_copy(
                su3[:, :, 128:136],
                swst[:, 0:nd * 8].rearrange("p (t h) -> p t h", h=8))

            def emit_self_mms(ps1s=ps1s, sust=sust, nd=nd):
                for di in range(nd):
                    nc.tensor.matmul(ps1s[di][:], lhsT=ident[:],
                                     rhs=sust[:, di * 136:(di + 1) * 136],
                                     start=True, stop=False)

            pending = []  # blocks whose scatter matmuls not yet emitted

            def emit_scatter(bi):
                s, bt = pending[bi]
                blk = blocks[s]
                for (di, pt_lo, pt_hi, lo, hi) in blk["passes"]:
                    for pt in range(pt_lo, pt_hi):
                        pp_off = bt["pp_of"][(di, pt)]
                        nc.tensor.matmul(
                            ps1s[di][:],
                            lhsT=bt["ind"][:, pp_off * 128:(pp_off + 1) * 128],
                            rhs=bt["ust"][:, pt * 136:(pt + 1) * 136],
                            start=False,
                            stop=(last_mm[di] == (s, pt, lo)))

            for s in range(NSCH):
                blk = blocks[s]
                blk["s"] = s
                blk["i1g"] = i1g
                blk["b16"] = blk["tbase"] * 8 - b16_0
                blk["ppg"] = blk["ppbase"] - pp0
                if blk["T"] == 0:
                    continue
                dlptp = sbp.tile([128, TPMAX * 128], U8, tag="dlptp",
                                 name="dlptp")
                TPb = sum(p[2] - p[1] for p in blk["passes"])
                nc.sync.dma_start(
                    dlptp[:, 0:TPb * 128],
                    DLPT[:, blk["ppbase"] * 128:(blk["ppbase"] + TPb) * 128]
                    .to_broadcast([128, TPb * 128]))
                bt = _emit_block_common(nc, pools, blk, i1g, dlpg, dlptp,
                                        iot, ioc, gather_fn, ROW1)
                T = blk["T"]
                # pass-tile offsets within this block's pass stream
                pp_of = {}
                pp = 0
                for (di, pt_lo, pt_hi, lo, hi) in blk["passes"]:
                    for pt in range(pt_lo, pt_hi):
                        pp_of[(di, pt)] = pp
                        pp += 1
                bt["pp_of"] = pp_of

                # adst via one-hot-transpose matmuls: psAD[lane, t*8+h]
                psAD = psadp.tile([128, TMAX * 8], F32, tag="psAD",
                                  name="psAD")
                covered = {}
                plist = [(di, pt) for (di, pt_lo, pt_hi, lo, hi)
                         in blk["passes"] for pt in range(pt_lo, pt_hi)]
                cover_count = {}
                for di, pt in plist:
                    cover_count[pt] = cover_count.get(pt, 0) + 1
                seen = {}
                for di, pt in plist:
                    seen[pt] = seen.get(pt, 0) + 1
                    pp_off = pp_of[(di, pt)]
                    nc.tensor.matmul(
                        psAD[:, pt * 8:(pt + 1) * 8],
                        lhsT=bt["indT"][:, pp_off * 128:(pp_off + 1) * 128],
                        rhs=adrg[:, di * 8:(di + 1) * 8],
                        start=(seen[pt] == 1),
                        stop=(seen[pt] == cover_count[pt]))
                adc = sb3.tile([128, TMAX * 8], F16, tag="adc", name="adc")
                nc.vector.tensor_copy(adc[:, 0:T * 8], psAD[:, 0:T * 8])

                g13 = bt["g"][:, 0:T * 136].rearrange("p (t e) -> p t e",
                                                      e=136)
                att = sb3.tile([128, TMAX * 8], F16, tag="att", name="att")
                at3 = att[:, 0:T * 8].rearrange("p (t h) -> p t h", h=8)
                nc.vector.tensor_tensor(
                    out=at3, in0=g13[:, :, 128:136],
                    in1=adc[:, 0:T * 8].rearrange("p (t h) -> p t h", h=8),
                    op=aadd)
                nc.vector.scalar_tensor_tensor(
                    out=at3, in0=at3, scalar=NEG_SLOPE, in1=at3,
                    op0=mult, op1=amax)
                wst = sb3.tile([128, TMAX * 8], F16, tag="wst", name="wst")
                nc.scalar.activation(out=wst[:, 0:T * 8], in_=att[:, 0:T * 8],
                                     func=AF.Exp)
                ust = sb3.tile([128, TMAX * 136], F16, tag="ust", name="ust")
                nc.vector.tensor_tensor(
                    out=ust[:, 0:T * 136]
                    .rearrange("p (t e) -> p t e", e=136)[:, :, 0:128]
                    .rearrange("p t (h c) -> p t h c", c=NHID),
                    in0=g13[:, :, 0:128]
                    .rearrange("p t (h c) -> p t h c", c=NHID),
                    in1=wst[:, 0:T * 8]
                    .rearrange("p (t h c) -> p t h c", h=8, c=1)
                    .to_broadcast([128, T, 8, NHID]),
                    op=mult)
                nc.vector.tensor_copy(
                    ust[:, 0:T * 136]
                    .rearrange("p (t e) -> p t e", e=136)[:, :, 128:136],
                    wst[:, 0:T * 8].rearrange("p (t h) -> p t h", h=8))
                bt["ust"] = ust
                pending.append((s, bt))
                if len(pending) == 1:
                    # previous group's post fills the PE bubble here, then
                    # this group's self-loop matmuls open the ps1 banks
                    if post_prev[0] is not None:
                        post_prev[0]()
                        post_prev[0] = None
                    emit_self_mms()
                # software-pipeline: emit scatter for the previous block
                if len(pending) >= 2:
                    emit_scatter(len(pending) - 2)
            emit_scatter(len(pending) - 1)

            # ---- group post-processing (deferred one group) ----
            def make_post(ds=ds, nd=nd, ps1s=ps1s):
              def post():
                o1g = sb2.tile([128, G * 136], F32, tag="o1g", name="o1g")
                for di in range(nd):
                    nc.vector.tensor_copy(o1g[:, di * 136:(di + 1) * 136],
                                          ps1s[di][:])
                o3 = o1g[:, 0:nd * 136].rearrange("p (t e) -> p t e", e=136)
                rc = sb2.tile([128, G * 8], F32, tag="rc", name="rc")
                nc.vector.reciprocal(
                    rc[:, 0:nd * 8].rearrange("p (t h) -> p t h", h=8),
                    o3[:, :, 128:136])
                eli = sb2.tile([128, G * 128], F32, tag="eli", name="eli")
                el3 = eli[:, 0:nd * 128].rearrange("p (t e) -> p t e", e=128)
                nc.vector.tensor_tensor(
                    out=el3.rearrange("p t (h c) -> p t h c", c=NHID),
                    in0=o3[:, :, 0:128].rearrange("p t (h c) -> p t h c", c=NHID),
                    in1=rc[:, 0:nd * 8]
                    .rearrange("p (t h c) -> p t h c", h=8, c=1)
                    .to_broadcast([128, nd, 8, NHID]),
                    op=mult)
                nc.vector.tensor_tensor(
                    out=el3, in0=el3,
                    in1=b1t[:].rearrange("p (t e) -> p t e", t=1)
                    .to_broadcast([128, nd, 128]),
                    op=aadd)
                # elu = max(x,0) + (exp(min(x,0)) - 1)
                t1 = sb2.tile([128, G * 128], F32, tag="t1", name="t1")
                nc.vector.tensor_scalar_min(t1[:, 0:nd * 128], eli[:, 0:nd * 128],
                                            0.0)
                t2 = sb2.tile([128, G * 128], F32, tag="t2", name="t2")
                nc.scalar.activation(out=t2[:, 0:nd * 128], in_=t1[:, 0:nd * 128],
                                     func=AF.Exp)
                nc.vector.tensor_scalar_add(t2[:, 0:nd * 128], t2[:, 0:nd * 128],
                                            -1.0)
                nc.vector.tensor_scalar_max(t1[:, 0:nd * 128], eli[:, 0:nd * 128],
                                            0.0)
                elu = sb2.tile([128, G * 128], F16, tag="elu", name="elu")
                nc.vector.tensor_tensor(out=elu[:, 0:nd * 128],
                                        in0=t1[:, 0:nd * 128],
                                        in1=t2[:, 0:nd * 128], op=aadd)
                h2rg = sb2.tile([128, G * 42], F16, tag="h2rg", name="h2rg")
                adw = sb2.tile([128, G], F16, tag="adw", name="adw")
                for di in range(nd):
                    psT = pstp.tile([128, 128], F16, tag="psT", name="psT")
                    nc.tensor.transpose(psT[:], elu[:, di * 128:(di + 1) * 128],
                                        ident[:])
                    eluT = sbp.tile([128, 128], F16, tag="eluT", name="eluT")
                    nc.vector.tensor_copy(eluT[:], psT[:])
                    ps2a = ps2p.tile([128, 42], F32, tag="ps2a", name="ps2a")
                    nc.tensor.matmul(ps2a[:], lhsT=eluT[:], rhs=w2s[:],
                                     start=True, stop=True)
                    # tab2 row: [h2 40 | asrc2 | one]  (one at col 41)
                    nc.vector.tensor_copy(h2rg[:, di * 42:di * 42 + 41],
                                          ps2a[:, 0:41])
                    nc.vector.tensor_copy(adw[:, di:di + 1], ps2a[:, 41:42])
                nc.vector.memset(
                    h2rg[:, 0:nd * 42]
                    .rearrange("p (t e) -> p t e", e=42)[:, :, 41:42], 1.0)
                r0 = ds[0] * 128
                r1 = (ds[-1] + 1) * 128
                nc.sync.dma_start(
                    tab2_sh[r0:r1, 0:42].rearrange("(t p) e -> p t e", p=128),
                    h2rg[:, 0:nd * 42].rearrange("p (t e) -> p t e", e=42))
                nc.sync.dma_start(
                    adr2[r0:r1, :].rearrange("(t p) e -> p t e", p=128),
                    adw[:, 0:nd].rearrange("p (t e) -> p t e", e=1))
              return post
            post_prev[0] = make_post()
        if post_prev[0] is not None:
            post_prev[0]()
            post_prev[0] = None


def _phase_e(nc, tc, layout, TMAX, TPMAX, TPGMAX, L16GMAX,
             IDX1, DLP, DLPT, IOTA, IOTC, B2, tab2, tab2_sh, adr2, OUT,
             clevel=9, bufs3=3, bufsp=2, sp=False):
    eq = mybir.AluOpType.is_equal
    mult = mybir.AluOpType.mult
    amax = mybir.AluOpType.max
    aadd = mybir.AluOpType.add
    sub = mybir.AluOpType.subtract
    AF = mybir.ActivationFunctionType

    with tc.tile_pool(name="sbE", bufs=1) as sbe, \
         tc.tile_pool(name="sbE2", bufs=3) as sb2, \
         tc.tile_pool(name="sbE3", bufs=bufs3) as sb3, \
         tc.tile_pool(name="sbEp", bufs=bufsp) as sbp, \
         tc.tile_pool(name="psE1", bufs=1, space="PSUM") as ps1p, \
         tc.tile_pool(name="psEad", bufs=2, space="PSUM") as psadp:
        iot = sbe.tile([128, 128], U8, tag="iotaE", name="iote")
        nc.sync.dma_start(iot[:], IOTA[:])
        ioc = sbe.tile([128, 128], U8, tag="iotcE", name="ioce")
        nc.sync.dma_start(ioc[:], IOTC[:])
        b2t = sbe.tile([128, NCLASS], F32, tag="b2t", name="b2t")
        nc.sync.dma_start(b2t[:], B2[:].to_broadcast([128, NCLASS]))
        from concourse.masks import make_identity
        identE = sbe.tile([128, 128], F16, tag="identE", name="identE")
        make_identity(nc, identE[:])

        pools = dict(sb3=sb3, TPMAX=TPMAX)

        def gather_fn(blk):
            g2 = sb3.tile([128, TMAX * 42], F16, tag="g2", name="g2")
            _dma_gather_raw(
                nc.gpsimd,
                g2[:, 0:blk["T"] * 42].rearrange("p (t e) -> p t e", e=42),
                tab2[blk["s"] * SCHW:(blk["s"] + 1) * SCHW, :],
                blk["i1g"][:, blk["b16"]:blk["b16"] + blk["L"] // 16],
                blk["L"], 42, ROW2, queue_num=blk["s"],
                single_packet=sp)
            return g2

        for g in range(NG):
            gr = layout.groups[g]
            ds = gr["ds"]
            nd = len(ds)
            blocks = gr["blocks"]
            g16 = sum(b["L"] for b in blocks) // 16
            b16_0 = blocks[0]["tbase"] * 8
            pp0 = blocks[0]["ppbase"]
            tpg = sum(sum(p[2] - p[1] for p in b["passes"]) for b in blocks)

            i1g = sb2.tile([128, L16GMAX], I16, tag="i1ge", name="i1ge")
            nc.sync.dma_start(i1g[:, 0:g16], IDX1[:, b16_0:b16_0 + g16])
            dlpg = sb2.tile([128, TPGMAX], U8, tag="dlpge", name="dlpge")
            nc.sync.dma_start(dlpg[:, 0:tpg], DLP[:, pp0:pp0 + tpg])
            ad2g = sb2.tile([128, G], F16, tag="ad2g", name="ad2g")
            nc.sync.dma_start(
                ad2g[:, 0:nd].rearrange("p (t e) -> p t e", e=1),
                adr2[ds[0] * 128:(ds[-1] + 1) * 128, :]
                .rearrange("(t p) e -> p t e", p=128))

            first_mm = {}
            last_mm = {}
            for s in range(NSCH):
                for (di, pt_lo, pt_hi, lo, hi) in blocks[s]["passes"]:
                    for pt in range(pt_lo, pt_hi):
                        if di not in first_mm:
                            first_mm[di] = (s, pt, lo)
                        last_mm[di] = (s, pt, lo)

            ps2s = {di: ps1p.tile([128, 42], F32, tag=f"ps2_{di}",
                                  name=f"ps2_{di}")
                    for di in range(nd)}

            # ---- self-loop path: local rows, identity scatter ----
            slr2 = sb2.tile([128, G * 42], F16, tag="slr2", name="slr2")
            nc.sync.dma_start(
                slr2[:, 0:nd * 42].rearrange("p (t e) -> p t e", e=42),
                tab2_sh[ds[0] * 128:(ds[-1] + 1) * 128, 0:42]
                .rearrange("(t p) e -> p t e", p=128))
            sl23 = slr2[:, 0:nd * 42].rearrange("p (t e) -> p t e", e=42)
            sat2 = sb2.tile([128, G], F16, tag="sat2", name="sat2")
            s2t3 = sat2[:, 0:nd].rearrange("p (t h) -> p t h", h=1)
            nc.vector.tensor_tensor(
                out=s2t3, in0=sl23[:, :, 40:41],
                in1=ad2g[:, 0:nd].rearrange("p (t h) -> p t h", h=1),
                op=aadd)
            nc.vector.scalar_tensor_tensor(
                out=s2t3, in0=s2t3, scalar=NEG_SLOPE, in1=s2t3,
                op0=mult, op1=amax)
            sw2 = sb2.tile([128, G], F16, tag="sw2", name="sw2")
            nc.scalar.activation(out=sw2[:, 0:nd], in_=sat2[:, 0:nd],
                                 func=AF.Exp)
            sgw = sb2.tile([128, G * 42], F16, tag="sgw", name="sgw")
            nc.vector.tensor_tensor(
                out=sgw[:, 0:nd * 42].rearrange("p (t e) -> p t e", e=42),
                in0=sl23,
                in1=sw2[:, 0:nd].rearrange("p (t s) -> p t s", s=1)
                .to_broadcast([128, nd, 42]),
                op=mult)
            for di in range(nd):
                nc.tensor.matmul(ps2s[di][:], lhsT=identE[:],
                                 rhs=sgw[:, di * 42:(di + 1) * 42],
                                 start=True, stop=False)

            pending = []

            def emit_scatter(bi):
                s, bt = pending[bi]
                blk = blocks[s]
                for (di, pt_lo, pt_hi, lo, hi) in blk["passes"]:
                    for pt in range(pt_lo, pt_hi):
                        pp_off = bt["pp_of"][(di, pt)]
                        nc.tensor.matmul(
                            ps2s[di][:],
                            lhsT=bt["ind"][:, pp_off * 128:(pp_off + 1) * 128],
                            rhs=bt["gw"][:, pt * 42:(pt + 1) * 42],
                            start=False,
                            stop=(last_mm[di] == (s, pt, lo)))

            for s in range(NSCH):
                blk = blocks[s]
                blk["s"] = s
                blk["i1g"] = i1g
                blk["b16"] = blk["tbase"] * 8 - b16_0
                blk["ppg"] = blk["ppbase"] - pp0
                if blk["T"] == 0:
                    continue
                dlptp = sbp.tile([128, TPMAX * 128], U8, tag="dlptpe",
                                 name="dlptpe")
                TPb = sum(p[2] - p[1] for p in blk["passes"])
                nc.sync.dma_start(
                    dlptp[:, 0:TPb * 128],
                    DLPT[:, blk["ppbase"] * 128:(blk["ppbase"] + TPb) * 128]
                    .to_broadcast([128, TPb * 128]))
                bt = _emit_block_common(nc, pools, blk, i1g, dlpg, dlptp,
                                        iot, ioc, gather_fn, 42)
                T = blk["T"]
                pp_of = {}
                pp = 0
                for (di, pt_lo, pt_hi, lo, hi) in blk["passes"]:
                    for pt in range(pt_lo, pt_hi):
                        pp_of[(di, pt)] = pp
                        pp += 1
                bt["pp_of"] = pp_of

                adc = sb3.tile([128, TMAX], F16, tag="adc2", name="adc2")
                if clevel >= 1:
                    psAD = psadp.tile([128, TMAX], F32, tag="psAD2",
                                      name="psAD2")
                    plist = [(di, pt) for (di, pt_lo, pt_hi, lo, hi)
                             in blk["passes"] for pt in range(pt_lo, pt_hi)]
                    cover_count = {}
                    for di, pt in plist:
                        cover_count[pt] = cover_count.get(pt, 0) + 1
                    seen = {}
                    for di, pt in plist:
                        seen[pt] = seen.get(pt, 0) + 1
                        pp_off = pp_of[(di, pt)]
                        nc.tensor.matmul(
                            psAD[:, pt:pt + 1],
                            lhsT=bt["indT"][:, pp_off * 128:(pp_off + 1) * 128],
                            rhs=ad2g[:, di:di + 1],
                            start=(seen[pt] == 1),
                            stop=(seen[pt] == cover_count[pt]))
                    nc.vector.tensor_copy(adc[:, 0:T], psAD[:, 0:T])
                else:
                    nc.vector.memset(adc[:, 0:T], 0.0)

                g23 = bt["g"][:, 0:T * 42].rearrange("p (t e) -> p t e", e=42)
                at2 = sb3.tile([128, TMAX], F16, tag="at2", name="at2")
                at23 = at2[:, 0:T].rearrange("p (t h) -> p t h", h=1)
                nc.vector.tensor_tensor(
                    out=at23, in0=g23[:, :, 40:41],
                    in1=adc[:, 0:T].rearrange("p (t h) -> p t h", h=1),
                    op=aadd)
                nc.vector.scalar_tensor_tensor(
                    out=at23, in0=at23, scalar=NEG_SLOPE, in1=at23,
                    op0=mult, op1=amax)
                w2t = sb3.tile([128, TMAX], F16, tag="w2t", name="w2t")
                nc.scalar.activation(out=w2t[:, 0:T], in_=at2[:, 0:T],
                                     func=AF.Exp)
                gw = sb3.tile([128, TMAX * 42], F16, tag="gw", name="gw")
                nc.vector.tensor_tensor(
                    out=gw[:, 0:T * 42].rearrange("p (t e) -> p t e", e=42),
                    in0=g23,
                    in1=w2t[:, 0:T].rearrange("p (t s) -> p t s", s=1)
                    .to_broadcast([128, T, 42]),
                    op=mult)
                bt["gw"] = gw
                pending.append((s, bt))
                if len(pending) >= 2:
                    emit_scatter(len(pending) - 2)
            emit_scatter(len(pending) - 1)

            # ---- group post: normalize + log_softmax + write ----
            o2g = sb2.tile([128, G * 42], F32, tag="o2g", name="o2g")
            for di in range(nd):
                nc.vector.tensor_copy(o2g[:, di * 42:(di + 1) * 42],
                                      ps2s[di][:])
            o3 = o2g[:, 0:nd * 42].rearrange("p (t e) -> p t e", e=42)
            rc2 = sb2.tile([128, G], F32, tag="rc2", name="rc2")
            nc.vector.reciprocal(
                rc2[:, 0:nd].rearrange("p (t h) -> p t h", h=1),
                o3[:, :, 41:42])
            lg = sb2.tile([128, G * NCLASS], F32, tag="lg", name="lg")
            lg3 = lg[:, 0:nd * NCLASS].rearrange("p (t e) -> p t e", e=NCLASS)
            nc.vector.tensor_tensor(
                out=lg3, in0=o3[:, :, 0:NCLASS],
                in1=rc2[:, 0:nd].rearrange("p (t h) -> p t h", h=1)
                .to_broadcast([128, nd, NCLASS]),
                op=mult)
            nc.vector.tensor_tensor(
                out=lg3, in0=lg3,
                in1=b2t[:].rearrange("p (t e) -> p t e", t=1)
                .to_broadcast([128, nd, NCLASS]),
                op=aadd)
            ex = sb2.tile([128, NCLASS], F32, tag="ex", name="ex")
            smg = sb2.tile([128, G], F32, tag="smg", name="smg")
            for di in range(nd):
                nc.scalar.activation(
                    out=ex[:], in_=lg[:, di * NCLASS:(di + 1) * NCLASS],
                    func=AF.Exp, accum_out=smg[:, di:di + 1])
            lsg = sb2.tile([128, G], F32, tag="lsg", name="lsg")
            nc.scalar.activation(out=lsg[:, 0:nd], in_=smg[:, 0:nd],
                                 func=AF.Ln)
            fin = sb2.tile([128, G * NCLASS], F32, tag="fin", name="fin")
            nc.vector.tensor_tensor(
                out=fin[:, 0:nd * NCLASS]
                .rearrange("p (t e) -> p t e", e=NCLASS),
                in0=lg3,
                in1=lsg[:, 0:nd].rearrange("p (t h) -> p t h", h=1)
                .to_broadcast([128, nd, NCLASS]),
                op=sub)
            r0 = ds[0] * 128
            r1 = (ds[-1] + 1) * 128
            nc.sync.dma_start(
                OUT[r0:r1, :].rearrange("(t p) e -> p t e", p=128),
                fin[:, 0:nd * NCLASS].rearrange("p (t e) -> p t e", e=NCLASS))


_CACHE = {}


def kernel(x, edge_index, W1, att_src1, att_dst1, b1, W2, att_src2, att_dst2, b2):
    x = np.asarray(x, dtype=np.float32)
    edge_index = np.asarray(edge_index)
    in_maps, layout = _prep(np.asarray(x), edge_index,
                            np.asarray(W1), np.asarray(att_src1),
                            np.asarray(att_dst1), np.asarray(W2),
                            np.asarray(att_src2), np.asarray(att_dst2),
                            b1=b1, b2=b2)
    key = layout.key()
    if key not in _CACHE:
        _CACHE[key] = _build(layout)
    nc = _CACHE[key]
    res = run_bass_kernel_spmd(nc, in_maps, core_ids=list(range(NCORES)))
    out = np.concatenate([res.results[k]["out"][:NPC] for k in range(NCORES)],
                         axis=0)
    return out.astype(np.float32)


# revision 23
# speedup vs baseline: 13.3358x; 1.0121x over previous
#!/usr/bin/env python3
"""2-layer GAT on 8 NeuronCores (Bass/Tile).

Sharding: nodes partitioned across 8 cores by dst id (graph parallel).
Layer tables (node features + attention scalars) are computed locally and
allgathered; per-edge source rows are fetched with dma_gather; per-edge
dst-attention values are computed with one-hot-transpose matmuls on the
tensor engine (no per-edge DMA gather); segment softmax/aggregation via
indicator matmuls.

Edge streams are grouped as (chunk-group g, src-window s) blocks so each
dma_gather instruction covers a whole block (~G chunks' edges for one
source window), minimizing GPSIMD descriptor-generation launches.
"""
import sys
import numpy as np

sys.path.insert(0, "/opt/pypackages")
sys.path.insert(0, "/opt/trn_rl_repo")

import concourse.bass as bass
import concourse.bacc as bacc
import concourse.tile as tile
import concourse.mybir as mybir
from concourse.bass_utils import run_bass_kernel_spmd

# problem constants
N = 100000
F_IN = 512
NHID = 16
HEADS = 8
NCLASS = 40
E = 1600000
NEG_SLOPE = 0.2

NCORES = 8
NPC = N // NCORES             # 12500 nodes per core
DCH = 128                     # dsts per chunk
NCH = (NPC + DCH - 1) // DCH  # 98 chunks
NPAD = NCH * DCH              # 12544 padded rows per core shard
NSCH = 4
SCHW = (NPAD * NCORES) // NSCH  # 25088 src rows per index window (int16-safe)
G = 5                         # chunks per group
NG = (NCH + G - 1) // G       # 20 groups

ROW1 = 256    # fp16 elems per L1 table row (512B): [h1 128 | asrc1 8 | pad]
ROW2 = 128    # fp16 elems per L2 table row (256B): [h2 40 | one | asrc2 | pad]

F16 = mybir.dt.float16
F32 = mybir.dt.float32
I16 = mybir.dt.int16
U8 = mybir.dt.uint8


def _wrap_block(v):
    """Wrap a 1-D int16 block (len % 16 == 0) into dma_gather idx layout
    [16, L/16], replicated to 128 partitions."""
    w = v.reshape(-1, 16).T
    return np.tile(w, (8, 1))


def _dma_gather_raw(gp, out_ap, in_ap, idxs_ap, num_idxs, elem_size, elem_step,
                    queue_num=0, single_packet=False):
    """dma_gather allowing elem_size (bytes read per row) that is not a
    multiple of 256B; the table row stride (elem_step) still must be."""
    from concourse.bass import exact_div
    stride_bytes = elem_step * mybir.dt.size(in_ap.dtype)
    stride_bytes_256 = exact_div(stride_bytes, 256)
    _in_ap = gp.lower_ap_dma(in_ap, for_custom_bir_dma=True)
    _idxs_ap = gp.lower_ap(idxs_ap)
    _out_ap = gp.lower_ap(out_ap)
    return gp.add_instruction(
        mybir.InstDMAGatherAnt(
            name=gp.bass.get_next_instruction_name(),
            ins=[*_in_ap, _idxs_ap, gp.lower_val_access(gp.to_reg(num_idxs))],
            outs=[_out_ap],
            transpose=False, num_idxs=num_idxs, elem_size=elem_size,
            stride_bytes_256=stride_bytes_256, gen_mode=0,
            single_packet=single_packet, queue_num=queue_num,
            sbuf_tokens_per_rank=0, sbuf_free_dim_per_rank=0,
            sbuf_free_dim_pad_per_rank=0, sbuf_byte_offset=0))


class _Layout:
    """Static (core-independent) edge-stream layout shared by all cores."""

    def __init__(self, cl16):
        # cl16: [NCH, NSCH] 16-aligned common cell lengths (block-padded)
        self.cl16 = cl16
        self.groups = []  # per g: dict with per-block and schedule info
        tbase = 0   # global tile base
        ppbase = 0  # global pass-tile base
        for g in range(NG):
            ds = list(range(g * G, min((g + 1) * G, NCH)))
            blocks = []
            for s in range(NSCH):
                cello = {}
                off = 0
                for d in ds:
                    cello[d] = off
                    off += int(cl16[d, s])
                L = off
                assert L % 128 == 0
                T = L // 128
                # passes: (di, pt_lo, pt_hi, lo, hi)
                passes = []
                for i, d in enumerate(ds):
                    cl = int(cl16[d, s])
                    if cl == 0:
                        continue
                    lo, hi = cello[d], cello[d] + cl
                    passes.append((i, lo // 128, (hi + 127) // 128, lo, hi))
                TP = sum(p[2] - p[1] for p in passes)
                blocks.append(dict(L=L, T=T, tbase=tbase, ppbase=ppbase,
                                   passes=passes, cello=cello))
                tbase += T
                ppbase += TP
            self.groups.append(dict(ds=ds, blocks=blocks))
        self.t_total = tbase
        self.tp_total = ppbase

    def key(self):
        return self.cl16.tobytes()


def _make_layout(cnt):
    """cnt: [NCORES, NCH, NSCH] per-cell edge counts -> _Layout."""
    cmax = cnt.max(axis=0)                                   # [NCH, NSCH]
    cl16 = ((cmax + 15) // 16 * 16).astype(np.int64)
    # round each (g, s) block up to a 128 multiple by padding its last
    # nonempty cell (ensures gathers cover whole tiles, no stale lanes)
    for g in range(NG):
        ds = list(range(g * G, min((g + 1) * G, NCH)))
        for s in range(NSCH):
            L = int(cl16[ds, s].sum())
            pad = (-L) % 128
            if pad:
                nz = [d for d in ds if cl16[d, s] > 0]
                tgt = nz[-1] if nz else ds[-1]
                cl16[tgt, s] += pad
    return _Layout(cl16)


def _prep(x, edge_index, W1, att_src1, att_dst1, W2, att_src2, att_dst2,
          b1=None, b2=None):
    """Host-side sharding/packing. Returns (in_maps, layout)."""
    # self-loops are handled by a dedicated local path (no gather, identity
    # scatter); only the real edges go through the gather streams
    src = np.asarray(edge_index[0], np.int64)
    dst = np.asarray(edge_index[1], np.int64)

    core = dst // NPC
    dl = (dst - core * NPC).astype(np.int64)       # local dst 0..12499
    d = dl >> 7                                    # dst chunk 0..97
    dlc = (dl & 127).astype(np.int64)              # slot within chunk
    s_pad = (src // NPC) * NPAD + (src % NPC)      # padded global src row
    sch = s_pad // SCHW
    sloc = (s_pad - sch * SCHW).astype(np.int64)   # 0..25087 (int16 ok)

    cell = ((core * NCH + d) * NSCH + sch).astype(np.int64)
    cnt = np.bincount(cell, minlength=NCORES * NCH * NSCH) \
            .reshape(NCORES, NCH, NSCH)
    layout = _make_layout(cnt)
    cl16 = layout.cl16

    # dense position of each edge: order within its cell by sloc
    order = np.argsort(cell * (SCHW + 1) + sloc, kind="stable")
    cell_s, sloc_s, dlc_s = cell[order], sloc[order], dlc[order]
    group_start = np.concatenate(
        [[0], np.cumsum(np.bincount(cell_s, minlength=NCORES * NCH * NSCH))])
    rank = np.arange(len(cell_s)) - group_start[cell_s]

    # cell start positions within the per-core stream
    cellstart = np.zeros((NCH, NSCH), np.int64)
    for g in range(NG):
        for s in range(NSCH):
            blk = layout.groups[g]["blocks"][s]
            for dd, off in blk["cello"].items():
                cellstart[dd, s] = blk["tbase"] * 128 + off
    cs_flat = cellstart.reshape(-1)  # [NCH*NSCH]
    cell_mod = cell_s % (NCH * NSCH)
    pos = cs_flat[cell_mod] + rank
    core_s = cell_s // (NCH * NSCH)

    t_total = layout.t_total
    Ltot = t_total * 128
    idx1 = np.zeros((NCORES, Ltot), dtype=np.int16)
    dlcD = np.full((NCORES, Ltot), 255, dtype=np.int16)
    idx1[core_s, pos] = sloc_s.astype(np.int16)
    dlcD[core_s, pos] = dlc_s.astype(np.int16)

    # wrapped idx stream (16-lane wrap, whole stream at once: every block is
    # 128-aligned so 16-alignment holds everywhere)
    IDX1 = np.zeros((NCORES, 128, Ltot // 16), dtype=np.int16)
    for k in range(NCORES):
        IDX1[k] = _wrap_block(idx1[k])

    # pass streams: DLP [128, TP] lane-major, DLPT [1, TP*128] flat
    TP = layout.tp_total
    DLP = np.full((NCORES, 128, TP), 255, dtype=np.uint8)
    DLPT = np.full((NCORES, 1, TP * 128), 255, dtype=np.uint8)
    for g in range(NG):
        for s in range(NSCH):
            blk = layout.groups[g]["blocks"][s]
            eb = blk["tbase"] * 128
            pp = blk["ppbase"]
            for (di, pt_lo, pt_hi, lo, hi) in blk["passes"]:
                for pt in range(pt_lo, pt_hi):
                    base = pt * 128
                    lanes = np.arange(base, base + 128)
                    vals = np.where(
                        (lanes >= lo) & (lanes < hi),
                        dlcD[:, eb + base:eb + base + 128], 255).astype(np.uint8)
                    DLP[:, :, pp] = vals
                    DLPT[:, 0, pp * 128:(pp + 1) * 128] = vals
                    pp += 1

    # weights
    asrc1 = att_src1.reshape(HEADS, NHID)
    adst1 = att_dst1.reshape(HEADS, NHID)
    W1r = W1.reshape(F_IN, HEADS, NHID)
    W1as = np.einsum("khc,hc->kh", W1r, asrc1)     # [512, 8]
    W1ad = np.einsum("khc,hc->kh", W1r, adst1)
    W1ext = np.concatenate([W1, W1as, W1ad], axis=1).astype(np.float16)  # [512, 144]
    W2as = W2 @ att_src2.reshape(NCLASS, 1)        # [128, 1]
    W2ad = W2 @ att_dst2.reshape(NCLASS, 1)
    W2ext = np.concatenate([W2, W2as, W2ad], axis=1).astype(np.float16)  # [128, 42]

    iota = np.broadcast_to(np.arange(128, dtype=np.uint8), (128, 128)).copy()
    iotc = np.broadcast_to(np.arange(128, dtype=np.uint8)[:, None],
                           (128, 128)).copy()

    in_maps = []
    for k in range(NCORES):
        xs = x[k * NPC:(k + 1) * NPC]              # [12500, 512]
        xT = np.zeros((F_IN, NPAD), dtype=np.float16)
        xT[:, :NPC] = xs.T
        in_maps.append({
            "xT": xT,
            "W1ext": W1ext,
            "W2ext": W2ext,
            "IDX1": IDX1[k],
            "DLP": DLP[k],
            "DLPT": DLPT[k],
            "iota": iota,
            "iotc": iotc,
            "B1": (np.zeros((1, 128), np.float32) if b1 is None
                   else np.asarray(b1, np.float32).reshape(1, 128)),
            "B2": (np.zeros((1, NCLASS), np.float32) if b2 is None
                   else np.asarray(b2, np.float32).reshape(1, NCLASS)),
        })
    return in_maps, layout


def _build(layout, phases="ABCDE", clevel=9, bufs3=4, bufsp=3,
           sp=False, split_coll=False):
    """Build the Bass module given the static layout."""
    from concourse.masks import make_identity

    t_total = layout.t_total
    tp_total = layout.tp_total
    TMAX = max(b["T"] for gr in layout.groups for b in gr["blocks"])
    TPMAX = max(sum(p[2] - p[1] for p in b["passes"])
                for gr in layout.groups for b in gr["blocks"])
    TPGMAX = max(sum(sum(p[2] - p[1] for p in b["passes"])
                     for b in gr["blocks"]) for gr in layout.groups)
    L16GMAX = max(sum(b["L"] for b in gr["blocks"]) // 16
                  for gr in layout.groups)

    nc = bacc.Bacc("TRN2", target_bir_lowering=False, debug=False,
                   enable_asserts=False, num_devices=NCORES,
                   num_swdge_queues=4)

    xT = nc.dram_tensor("xT", [F_IN, NPAD], F16, kind="ExternalInput")
    W1e = nc.dram_tensor("W1ext", [F_IN, 144], F16, kind="ExternalInput")
    W2e = nc.dram_tensor("W2ext", [128, 42], F16, kind="ExternalInput")
    IDX1 = nc.dram_tensor("IDX1", [128, t_total * 8], I16, kind="ExternalInput")
    DLP = nc.dram_tensor("DLP", [128, tp_total], U8, kind="ExternalInput")
    DLPT = nc.dram_tensor("DLPT", [1, tp_total * 128], U8,
                          kind="ExternalInput")
    IOTA = nc.dram_tensor("iota", [128, 128], U8, kind="ExternalInput")
    IOTC = nc.dram_tensor("iotc", [128, 128], U8, kind="ExternalInput")
    B1 = nc.dram_tensor("B1", [1, 128], F32, kind="ExternalInput")
    B2 = nc.dram_tensor("B2", [1, NCLASS], F32, kind="ExternalInput")
    OUT = nc.dram_tensor("out", [NPAD, NCLASS], F32, kind="ExternalOutput")

    tab1_sh = nc.dram_tensor("tab1_sh", [NPAD, ROW1], F16, kind="Internal")
    tab1 = nc.dram_tensor("tab1", [NPAD * NCORES, ROW1], F16, kind="Internal",
                          addr_space="Shared")
    tab2_sh = nc.dram_tensor("tab2_sh", [NPAD, ROW2], F16, kind="Internal")
    tab2 = nc.dram_tensor("tab2", [NPAD * NCORES, ROW2], F16, kind="Internal",
                          addr_space="Shared")
    adr1 = nc.dram_tensor("adr1", [NPAD, 8], F16, kind="Internal")
    adr2 = nc.dram_tensor("adr2", [NPAD, 1], F16, kind="Internal")

    with tile.TileContext(nc) as tc:
        if "A" in phases:
            _phase_a(nc, tc, xT, W1e, tab1_sh, adr1)
        if "B" in phases:
            if split_coll:
                H = NPAD // 2
                tab1v = tab1[:].rearrange("(c r) e -> c r e", c=NCORES)
                nc.gpsimd.collective_compute(
                    "AllGather", mybir.AluOpType.bypass,
                    replica_groups=[list(range(NCORES))],
                    ins=[tab1_sh[0:H, :]], outs=[tab1v[:, 0:H, :]])
                nc.gpsimd.collective_compute(
                    "AllGather", mybir.AluOpType.bypass,
                    replica_groups=[list(range(NCORES))],
                    ins=[tab1_sh[H:NPAD, :]], outs=[tab1v[:, H:NPAD, :]])
            else:
                nc.gpsimd.collective_compute(
                    "AllGather", mybir.AluOpType.bypass,
                    replica_groups=[list(range(NCORES))],
                    ins=[tab1_sh[:]], outs=[tab1[:]])
        if "C" in phases:
            _phase_c(nc, tc, layout, TMAX, TPMAX, TPGMAX, L16GMAX,
                     make_identity,
                     IDX1, DLP, DLPT, IOTA, IOTC, B1, W2e, tab1, tab1_sh,
                     adr1, tab2_sh, adr2, bufs3=bufs3, bufsp=bufsp, sp=sp)
        if "D" in phases:
            if split_coll:
                H = NPAD // 2
                tab2v = tab2[:].rearrange("(c r) e -> c r e", c=NCORES)
                nc.gpsimd.collective_compute(
                    "AllGather", mybir.AluOpType.bypass,
                    replica_groups=[list(range(NCORES))],
                    ins=[tab2_sh[0:H, :]], outs=[tab2v[:, 0:H, :]])
                nc.gpsimd.collective_compute(
                    "AllGather", mybir.AluOpType.bypass,
                    replica_groups=[list(range(NCORES))],
                    ins=[tab2_sh[H:NPAD, :]], outs=[tab2v[:, H:NPAD, :]])
            else:
                nc.gpsimd.collective_compute(
                    "AllGather", mybir.AluOpType.bypass,
                    replica_groups=[list(range(NCORES))],
                    ins=[tab2_sh[:]], outs=[tab2[:]])
        if "E" in phases:
            _phase_e(nc, tc, layout, TMAX, TPMAX, TPGMAX, L16GMAX,
                     IDX1, DLP, DLPT, IOTA, IOTC, B2, tab2, tab2_sh, adr2,
                     OUT, clevel=clevel, bufs3=bufs3, bufsp=bufsp, sp=sp)

    nc.compile()
    return nc


def _phase_a(nc, tc, xT, W1e, tab1_sh, adr1):
    with tc.tile_pool(name="sbA", bufs=1) as sba, \
         tc.tile_pool(name="sbA2", bufs=4) as sba2, \
         tc.tile_pool(name="psA", bufs=4, space="PSUM") as psa:
        xts = [sba.tile([128, NPAD], F16, tag=f"xt{k}", name=f"xt{k}")
               for k in range(4)]
        w1s = [sba.tile([128, 144], F16, tag=f"w1{k}", name=f"w1{k}")
               for k in range(4)]
        for k in range(4):
            nc.sync.dma_start(xts[k][:], xT[k * 128:(k + 1) * 128, :])
            nc.sync.dma_start(w1s[k][:], W1e[k * 128:(k + 1) * 128, :])
        for nt in range(NCH):
            ps = psa.tile([128, 144], F32, tag="psA", name="psA")
            for k in range(4):
                nc.tensor.matmul(ps[:], lhsT=xts[k][:, nt * 128:(nt + 1) * 128],
                                 rhs=w1s[k][:], start=(k == 0), stop=(k == 3))
            row = sba2.tile([128, 136], F16, tag="row", name="row")
            nc.vector.tensor_copy(row[:], ps[:, 0:136])
            nc.sync.dma_start(tab1_sh[nt * 128:(nt + 1) * 128, 0:136], row[:])
            t8 = sba2.tile([128, 8], F16, tag="t8", name="t8")
            nc.vector.tensor_copy(t8[:], ps[:, 136:144])
            nc.sync.dma_start(adr1[nt * 128:(nt + 1) * 128, :], t8[:])


def _emit_block_common(nc, pools, blk, i1g, dlpg, dlptp, iot, ioc,
                       gather_fn, elem):
    """Per-(g,s)-block: gather + ind/indT builds. Returns dict of tiles."""
    sb3 = pools["sb3"]
    T, TPb = blk["T"], sum(p[2] - p[1] for p in blk["passes"])
    if T == 0:
        return None
    gt = gather_fn(blk)

    eq = mybir.AluOpType.is_equal
    ind = sb3.tile([128, pools["TPMAX"] * 128], F16, tag="ind", name="ind")
    nc.vector.tensor_tensor(
        out=ind[:, 0:TPb * 128].rearrange("p (t s) -> p t s", s=128),
        in0=iot[:].rearrange("p (t s) -> p t s", t=1)
        .to_broadcast([128, TPb, 128]),
        in1=dlpg[:, blk["ppg"]:blk["ppg"] + TPb]
        .rearrange("p (t s) -> p t s", s=1).to_broadcast([128, TPb, 128]),
        op=eq)
    indT = sb3.tile([128, pools["TPMAX"] * 128], F16, tag="indT", name="indT")
    nc.vector.tensor_tensor(
        out=indT[:, 0:TPb * 128].rearrange("p (t s) -> p t s", s=128),
        in0=ioc[:].rearrange("p (t s) -> p t s", t=1)
        .to_broadcast([128, TPb, 128]),
        in1=dlptp[:, 0:TPb * 128].rearrange("p (t s) -> p t s", s=128),
        op=eq)
    return dict(g=gt, ind=ind, indT=indT, T=T, TPb=TPb)


def _phase_c(nc, tc, layout, TMAX, TPMAX, TPGMAX, L16GMAX, make_identity,
             IDX1, DLP, DLPT, IOTA, IOTC, B1, W2e, tab1, tab1_sh, adr1,
             tab2_sh, adr2, bufs3=3, bufsp=2, sp=False):
    eq = mybir.AluOpType.is_equal
    mult = mybir.AluOpType.mult
    amax = mybir.AluOpType.max
    aadd = mybir.AluOpType.add
    AF = mybir.ActivationFunctionType

    with tc.tile_pool(name="sbC", bufs=1) as sbc, \
         tc.tile_pool(name="sbC2", bufs=3) as sb2, \
         tc.tile_pool(name="sbC3", bufs=bufs3) as sb3, \
         tc.tile_pool(name="sbCp", bufs=bufsp) as sbp, \
         tc.tile_pool(name="psC1", bufs=1, space="PSUM") as ps1p, \
         tc.tile_pool(name="psCad", bufs=1, space="PSUM") as psadp, \
         tc.tile_pool(name="psCt", bufs=1, space="PSUM") as pstp, \
         tc.tile_pool(name="psC2", bufs=1, space="PSUM") as ps2p:
        iot = sbc.tile([128, 128], U8, tag="iota", name="iotc_")
        nc.sync.dma_start(iot[:], IOTA[:])
        ioc = sbc.tile([128, 128], U8, tag="iotc", name="iocc")
        nc.sync.dma_start(ioc[:], IOTC[:])
        ident = sbc.tile([128, 128], F16, tag="ident", name="ident")
        make_identity(nc, ident[:])
        w2s = sbc.tile([128, 42], F16, tag="w2s", name="w2s")
        nc.sync.dma_start(w2s[:], W2e[:])
        b1t = sbc.tile([128, 128], F32, tag="b1t", name="b1t")
        nc.sync.dma_start(b1t[:], B1[:].to_broadcast([128, 128]))

        pools = dict(sb3=sb3, TPMAX=TPMAX)
        post_prev = [None]

        def gather_fn(blk):
            g1 = sb3.tile([128, TMAX * 136], F16, tag="g1", name="g1")
            _dma_gather_raw(
                nc.gpsimd,
                g1[:, 0:blk["T"] * 136].rearrange("p (t e) -> p t e", e=136),
                tab1[blk["s"] * SCHW:(blk["s"] + 1) * SCHW, :],
                blk["i1g"][:, blk["b16"]:blk["b16"] + blk["L"] // 16],
                blk["L"], 136, ROW1, queue_num=blk["s"],
                single_packet=sp)
            return g1

        for g in range(NG):
            gr = layout.groups[g]
            ds = gr["ds"]
            nd = len(ds)
            blocks = gr["blocks"]
            g16 = sum(b["L"] for b in blocks) // 16
            b16_0 = blocks[0]["tbase"] * 8  # = tbase*128/16
            pp0 = blocks[0]["ppbase"]
            tpg = sum(sum(p[2] - p[1] for p in b["passes"]) for b in blocks)

            i1g = sb2.tile([128, L16GMAX], I16, tag="i1g", name="i1g")
            nc.sync.dma_start(i1g[:, 0:g16], IDX1[:, b16_0:b16_0 + g16])
            dlpg = sb2.tile([128, TPGMAX], U8, tag="dlpg", name="dlpg")
            nc.sync.dma_start(dlpg[:, 0:tpg], DLP[:, pp0:pp0 + tpg])
            adrg = sb2.tile([128, G * 8], F16, tag="adrg", name="adrg")
            nc.sync.dma_start(
                adrg[:, 0:nd * 8].rearrange("p (t h) -> p t h", h=8),
                adr1[ds[0] * 128:(ds[-1] + 1) * 128, :]
                .rearrange("(t p) h -> p t h", p=128))

            # chunk first/last (pass, tile) over the whole group
            first_mm = {}
            last_mm = {}
            for s in range(NSCH):
                for (di, pt_lo, pt_hi, lo, hi) in blocks[s]["passes"]:
                    for pt in range(pt_lo, pt_hi):
                        if di not in first_mm:
                            first_mm[di] = (s, pt, lo)
                        last_mm[di] = (s, pt, lo)

            ps1s = {di: ps1p.tile([128, 136], F32, tag=f"ps1_{di}",
                                  name=f"ps1_{di}")
                    for di in range(nd)}

            # ---- self-loop path: local rows, identity scatter ----
            slr = sb2.tile([128, G * 136], F16, tag="slr", name="slr")
            nc.sync.dma_start(
                slr[:, 0:nd * 136].rearrange("p (t e) -> p t e", e=136),
                tab1_sh[ds[0] * 128:(ds[-1] + 1) * 128, 0:136]
                .rearrange("(t p) e -> p t e", p=128))
            slr3 = slr[:, 0:nd * 136].rearrange("p (t e) -> p t e", e=136)
            sat = sb2.tile([128, G * 8], F16, tag="sat", name="sat")
            sat3 = sat[:, 0:nd * 8].rearrange("p (t h) -> p t h", h=8)
            nc.vector.tensor_tensor(
                out=sat3, in0=slr3[:, :, 128:136],
                in1=adrg[:, 0:nd * 8].rearrange("p (t h) -> p t h", h=8),
                op=aadd)
            nc.vector.scalar_tensor_tensor(
                out=sat3, in0=sat3, scalar=NEG_SLOPE, in1=sat3,
                op0=mult, op1=amax)
            swst = sb2.tile([128, G * 8], F16, tag="swst", name="swst")
            nc.scalar.activation(out=swst[:, 0:nd * 8], in_=sat[:, 0:nd * 8],
                                 func=AF.Exp)
            sust = sb2.tile([128, G * 136], F16, tag="sust", name="sust")
            su3 = sust[:, 0:nd * 136].rearrange("p (t e) -> p t e", e=136)
            nc.vector.tensor_tensor(
                out=su3[:, :, 0:128].rearrange("p t (h c) -> p t h c", c=NHID),
                in0=slr3[:, :, 0:128].rearrange("p t (h c) -> p t h c", c=NHID),
                in1=swst[:, 0:nd * 8]
                .rearrange("p (t h c) -> p t h c", h=8, c=1)
                .to_broadcast([128, nd, 8, NHID]),
                op=mult)
            nc.vector.tensor_copy(
                su3[:, :, 128:136],
                swst[:, 0:nd * 8].rearrange("p (t h) -> p t h", h=8))

            def emit_self_mms(ps1s=ps1s, sust=sust, nd=nd):
                for di in range(nd):
                    nc.tensor.matmul(ps1s[di][:], lhsT=ident[:],
                                     rhs=sust[:, di * 136:(di + 1) * 136],
                                     start=True, stop=False)

            pending = []  # blocks whose scatter matmuls not yet emitted

            def emit_scatter(bi):
                s, bt = pending[bi]
                blk = blocks[s]
                for (di, pt_lo, pt_hi, lo, hi) in blk["passes"]:
                    for pt in range(pt_lo, pt_hi):
                        pp_off = bt["pp_of"][(di, pt)]
                        nc.tensor.matmul(
                            ps1s[di][:],
                            lhsT=bt["ind"][:, pp_off * 128:(pp_off + 1) * 128],
                            rhs=bt["ust"][:, pt * 136:(pt + 1) * 136],
                            start=False,
                            stop=(last_mm[di] == (s, pt, lo)))

            for s in range(NSCH):
                blk = blocks[s]
                blk["s"] = s
                blk["i1g"] = i1g
                blk["b16"] = blk["tbase"] * 8 - b16_0
                blk["ppg"] = blk["ppbase"] - pp0
                if blk["T"] == 0:
                    continue
                dlptp = sbp.tile([128, TPMAX * 128], U8, tag="dlptp",
                                 name="dlptp")
                TPb = sum(p[2] - p[1] for p in blk["passes"])
                nc.sync.dma_start(
                    dlptp[:, 0:TPb * 128],
                    DLPT[:, blk["ppbase"] * 128:(blk["ppbase"] + TPb) * 128]
                    .to_broadcast([128, TPb * 128]))
                bt = _emit_block_common(nc, pools, blk, i1g, dlpg, dlptp,
                                        iot, ioc, gather_fn, ROW1)
                T = blk["T"]
                # pass-tile offsets within this block's pass stream
                pp_of = {}
                pp = 0
                for (di, pt_lo, pt_hi, lo, hi) in blk["passes"]:
                    for pt in range(pt_lo, pt_hi):
                        pp_of[(di, pt)] = pp
                        pp += 1
                bt["pp_of"] = pp_of

                # adst via one-hot-transpose matmuls: psAD[lane, t*8+h]
                psAD = psadp.tile([128, TMAX * 8], F32, tag="psAD",
                                  name="psAD")
                covered = {}
                plist = [(di, pt) for (di, pt_lo, pt_hi, lo, hi)
                         in blk["passes"] for pt in range(pt_lo, pt_hi)]
                cover_count = {}
                for di, pt in plist:
                    cover_count[pt] = cover_count.get(pt, 0) + 1
                seen = {}
                for di, pt in plist:
                    seen[pt] = seen.get(pt, 0) + 1
                    pp_off = pp_of[(di, pt)]
                    nc.tensor.matmul(
                        psAD[:, pt * 8:(pt + 1) * 8],
                        lhsT=bt["indT"][:, pp_off * 128:(pp_off + 1) * 128],
                        rhs=adrg[:, di * 8:(di + 1) * 8],
                        start=(seen[pt] == 1),
                        stop=(seen[pt] == cover_count[pt]))
                adc = sb3.tile([128, TMAX * 8], F16, tag="adc", name="adc")
                nc.vector.tensor_copy(adc[:, 0:T * 8], psAD[:, 0:T * 8])

                g13 = bt["g"][:, 0:T * 136].rearrange("p (t e) -> p t e",
                                                      e=136)
                att = sb3.tile([128, TMAX * 8], F16, tag="att", name="att")
                at3 = att[:, 0:T * 8].rearrange("p (t h) -> p t h", h=8)
                nc.vector.tensor_tensor(
                    out=at3, in0=g13[:, :, 128:136],
                    in1=adc[:, 0:T * 8].rearrange("p (t h) -> p t h", h=8),
                    op=aadd)
                nc.vector.scalar_tensor_tensor(
                    out=at3, in0=at3, scalar=NEG_SLOPE, in1=at3,
                    op0=mult, op1=amax)
                wst = sb3.tile([128, TMAX * 8], F16, tag="wst", name="wst")
                nc.scalar.activation(out=wst[:, 0:T * 8], in_=att[:, 0:T * 8],
                                     func=AF.Exp)
                ust = sb3.tile([128, TMAX * 136], F16, tag="ust", name="ust")
                nc.vector.tensor_tensor(
                    out=ust[:, 0:T * 136]
                    .rearrange("p (t e) -> p t e", e=136)[:, :, 0:128]
                    .rearrange("p t (h c) -> p t h c", c=NHID),
                    in0=g13[:, :, 0:128]
                    .rearrange("p t (h c) -> p t h c", c=NHID),
                    in1=wst[:, 0:T * 8]
                    .rearrange("p (t h c) -> p t h c", h=8, c=1)
                    .to_broadcast([128, T, 8, NHID]),
                    op=mult)
                nc.vector.tensor_copy(
                    ust[:, 0:T * 136]
                    .rearrange("p (t e) -> p t e", e=136)[:, :, 128:136],
                    wst[:, 0:T * 8].rearrange("p (t h) -> p t h", h=8))
                bt["ust"] = ust
                pending.append((s, bt))
                if len(pending) == 1:
                    # previous group's post fills the PE bubble here, then
                    # this group's self-loop matmuls open the ps1 banks
                    if post_prev[0] is not None:
                        post_prev[0]()
                        post_prev[0] = None
                    emit_self_mms()
                # software-pipeline: emit scatter for the previous block
                if len(pending) >= 2:
                    emit_scatter(len(pending) - 2)
            emit_scatter(len(pending) - 1)

            # ---- group post-processing (deferred one group) ----
            def make_post(ds=ds, nd=nd, ps1s=ps1s):
              def post():
                o1g = sb2.tile([128, G * 136], F32, tag="o1g", name="o1g")
                for di in range(nd):
                    nc.vector.tensor_copy(o1g[:, di * 136:(di + 1) * 136],
                                          ps1s[di][:])
                o3 = o1g[:, 0:nd * 136].rearrange("p (t e) -> p t e", e=136)
                rc = sb2.tile([128, G * 8], F32, tag="rc", name="rc")
                nc.vector.reciprocal(
                    rc[:, 0:nd * 8].rearrange("p (t h) -> p t h", h=8),
                    o3[:, :, 128:136])
                eli = sb2.tile([128, G * 128], F32, tag="eli", name="eli")
                el3 = eli[:, 0:nd * 128].rearrange("p (t e) -> p t e", e=128)
                nc.vector.tensor_tensor(
                    out=el3.rearrange("p t (h c) -> p t h c", c=NHID),
                    in0=o3[:, :, 0:128].rearrange("p t (h c) -> p t h c", c=NHID),
                    in1=rc[:, 0:nd * 8]
                    .rearrange("p (t h c) -> p t h c", h=8, c=1)
                    .to_broadcast([128, nd, 8, NHID]),
                    op=mult)
                nc.vector.tensor_tensor(
                    out=el3, in0=el3,
                    in1=b1t[:].rearrange("p (t e) -> p t e", t=1)
                    .to_broadcast([128, nd, 128]),
                    op=aadd)
                # elu = max(x,0) + (exp(min(x,0)) - 1)
                t1 = sb2.tile([128, G * 128], F32, tag="t1", name="t1")
                nc.vector.tensor_scalar_min(t1[:, 0:nd * 128], eli[:, 0:nd * 128],
                                            0.0)
                t2 = sb2.tile([128, G * 128], F32, tag="t2", name="t2")
                nc.scalar.activation(out=t2[:, 0:nd * 128], in_=t1[:, 0:nd * 128],
                                     func=AF.Exp)
                nc.vector.tensor_scalar_add(t2[:, 0:nd * 128], t2[:, 0:nd * 128],
                                            -1.0)
                nc.vector.tensor_scalar_max(t1[:, 0:nd * 128], eli[:, 0:nd * 128],
                                            0.0)
                elu = sb2.tile([128, G * 128], F16, tag="elu", name="elu")
                nc.vector.tensor_tensor(out=elu[:, 0:nd * 128],
                                        in0=t1[:, 0:nd * 128],
                                        in1=t2[:, 0:nd * 128], op=aadd)
                h2rg = sb2.tile([128, G * 42], F16, tag="h2rg", name="h2rg")
                adw = sb2.tile([128, G], F16, tag="adw", name="adw")
                for di in range(nd):
                    psT = pstp.tile([128, 128], F16, tag="psT", name="psT")
                    nc.tensor.transpose(psT[:], elu[:, di * 128:(di + 1) * 128],
                                        ident[:])
                    eluT = sbp.tile([128, 128], F16, tag="eluT", name="eluT")
                    nc.vector.tensor_copy(eluT[:], psT[:])
                    ps2a = ps2p.tile([128, 42], F32, tag="ps2a", name="ps2a")
                    nc.tensor.matmul(ps2a[:], lhsT=eluT[:], rhs=w2s[:],
                                     start=True, stop=True)
                    # tab2 row: [h2 40 | asrc2 | one]  (one at col 41)
                    nc.vector.tensor_copy(h2rg[:, di * 42:di * 42 + 41],
                                          ps2a[:, 0:41])
                    nc.vector.tensor_copy(adw[:, di:di + 1], ps2a[:, 41:42])
                nc.vector.memset(
                    h2rg[:, 0:nd * 42]
                    .rearrange("p (t e) -> p t e", e=42)[:, :, 41:42], 1.0)
                r0 = ds[0] * 128
                r1 = (ds[-1] + 1) * 128
                nc.sync.dma_start(
                    tab2_sh[r0:r1, 0:42].rearrange("(t p) e -> p t e", p=128),
                    h2rg[:, 0:nd * 42].rearrange("p (t e) -> p t e", e=42))
                nc.sync.dma_start(
                    adr2[r0:r1, :].rearrange("(t p) e -> p t e", p=128),
                    adw[:, 0:nd].rearrange("p (t e) -> p t e", e=1))
              return post
            post_prev[0] = make_post()
        if post_prev[0] is not None:
            post_prev[0]()
            post_prev[0] = None


def _phase_e(nc, tc, layout, TMAX, TPMAX, TPGMAX, L16GMAX,
             IDX1, DLP, DLPT, IOTA, IOTC, B2, tab2, tab2_sh, adr2, OUT,
             clevel=9, bufs3=3, bufsp=2, sp=False):
    eq = mybir.AluOpType.is_equal
    mult = mybir.AluOpType.mult
    amax = mybir.AluOpType.max
    aadd = mybir.AluOpType.add
    sub = mybir.AluOpType.subtract
    AF = mybir.ActivationFunctionType

    with tc.tile_pool(name="sbE", bufs=1) as sbe, \
         tc.tile_pool(name="sbE2", bufs=3) as sb2, \
         tc.tile_pool(name="sbE3", bufs=bufs3) as sb3, \
         tc.tile_pool(name="sbEp", bufs=bufsp) as sbp, \
         tc.tile_pool(name="psE1", bufs=1, space="PSUM") as ps1p, \
         tc.tile_pool(name="psEad", bufs=2, space="PSUM") as psadp:
        iot = sbe.tile([128, 128], U8, tag="iotaE", name="iote")
        nc.sync.dma_start(iot[:], IOTA[:])
        ioc = sbe.tile([128, 128], U8, tag="iotcE", name="ioce")
        nc.sync.dma_start(ioc[:], IOTC[:])
        b2t = sbe.tile([128, NCLASS], F32, tag="b2t", name="b2t")
        nc.sync.dma_start(b2t[:], B2[:].to_broadcast([128, NCLASS]))
        from concourse.masks import make_identity
        identE = sbe.tile([128, 128], F16, tag="identE", name="identE")
        make_identity(nc, identE[:])

        pools = dict(sb3=sb3, TPMAX=TPMAX)

        def gather_fn(blk):
            g2 = sb3.tile([128, TMAX * 42], F16, tag="g2", name="g2")
            _dma_gather_raw(
                nc.gpsimd,
                g2[:, 0:blk["T"] * 42].rearrange("p (t e) -> p t e", e=42),
                tab2[blk["s"] * SCHW:(blk["s"] + 1) * SCHW, :],
                blk["i1g"][:, blk["b16"]:blk["b16"] + blk["L"] // 16],
                blk["L"], 42, ROW2, queue_num=blk["s"],
                single_packet=sp)
            return g2

        for g in range(NG):
            gr = layout.groups[g]
            ds = gr["ds"]
            nd = len(ds)
            blocks = gr["blocks"]
            g16 = sum(b["L"] for b in blocks) // 16
            b16_0 = blocks[0]["tbase"] * 8
            pp0 = blocks[0]["ppbase"]
            tpg = sum(sum(p[2] - p[1] for p in b["passes"]) for b in blocks)

            i1g = sb2.tile([128, L16GMAX], I16, tag="i1ge", name="i1ge")
            nc.sync.dma_start(i1g[:, 0:g16], IDX1[:, b16_0:b16_0 + g16])
            dlpg = sb2.tile([128, TPGMAX], U8, tag="dlpge", name="dlpge")
            nc.sync.dma_start(dlpg[:, 0:tpg], DLP[:, pp0:pp0 + tpg])
            ad2g = sb2.tile([128, G], F16, tag="ad2g", name="ad2g")
            nc.sync.dma_start(
                ad2g[:, 0:nd].rearrange("p (t e) -> p t e", e=1),
                adr2[ds[0] * 128:(ds[-1] + 1) * 128, :]
                .rearrange("(t p) e -> p t e", p=128))

            first_mm = {}
            last_mm = {}
            for s in range(NSCH):
                for (di, pt_lo, pt_hi, lo, hi) in blocks[s]["passes"]:
                    for pt in range(pt_lo, pt_hi):
                        if di not in first_mm:
                            first_mm[di] = (s, pt, lo)
                        last_mm[di] = (s, pt, lo)

            ps2s = {di: ps1p.tile([128, 42], F32, tag=f"ps2_{di}",
                                  name=f"ps2_{di}")
                    for di in range(nd)}

            # ---- self-loop path: local rows, identity scatter ----
            slr2 = sb2.tile([128, G * 42], F16, tag="slr2", name="slr2")
            nc.sync.dma_start(
                slr2[:, 0:nd * 42].rearrange("p (t e) -> p t e", e=42),
                tab2_sh[ds[0] * 128:(ds[-1] + 1) * 128, 0:42]
                .rearrange("(t p) e -> p t e", p=128))
            sl23 = slr2[:, 0:nd * 42].rearrange("p (t e) -> p t e", e=42)
            sat2 = sb2.tile([128, G], F16, tag="sat2", name="sat2")
            s2t3 = sat2[:, 0:nd].rearrange("p (t h) -> p t h", h=1)
            nc.vector.tensor_tensor(
                out=s2t3, in0=sl23[:, :, 40:41],
                in1=ad2g[:, 0:nd].rearrange("p (t h) -> p t h", h=1),
                op=aadd)
            nc.vector.scalar_tensor_tensor(
                out=s2t3, in0=s2t3, scalar=NEG_SLOPE, in1=s2t3,
                op0=mult, op1=amax)
            sw2 = sb2.tile([128, G], F16, tag="sw2", name="sw2")
            nc.scalar.activation(out=sw2[:, 0:nd], in_=sat2[:, 0:nd],
                                 func=AF.Exp)
            sgw = sb2.tile([128, G * 42], F16, tag="sgw", name="sgw")
            nc.vector.tensor_tensor(
                out=sgw[:, 0:nd * 42].rearrange("p (t e) -> p t e", e=42),
                in0=sl23,
                in1=sw2[:, 0:nd].rearrange("p (t s) -> p t s", s=1)
                .to_broadcast([128, nd, 42]),
                op=mult)
            for di in range(nd):
                nc.tensor.matmul(ps2s[di][:], lhsT=identE[:],
                                 rhs=sgw[:, di * 42:(di + 1) * 42],
                                 start=True, stop=False)

            pending = []

            def emit_scatter(bi):
                s, bt = pending[bi]
                blk = blocks[s]
                for (di, pt_lo, pt_hi, lo, hi) in blk["passes"]:
                    for pt in range(pt_lo, pt_hi):
                        pp_off = bt["pp_of"][(di, pt)]
                        nc.tensor.matmul(
                            ps2s[di][:],
                            lhsT=bt["ind"][:, pp_off * 128:(pp_off + 1) * 128],
                            rhs=bt["gw"][:, pt * 42:(pt + 1) * 42],
                            start=False,
                            stop=(last_mm[di] == (s, pt, lo)))

            for s in range(NSCH):
                blk = blocks[s]
                blk["s"] = s
                blk["i1g"] = i1g
                blk["b16"] = blk["tbase"] * 8 - b16_0
                blk["ppg"] = blk["ppbase"] - pp0
                if blk["T"] == 0:
                    continue
                dlptp = sbp.tile([128, TPMAX * 128], U8, tag="dlptpe",
                                 name="dlptpe")
                TPb = sum(p[2] - p[1] for p in blk["passes"])
                nc.sync.dma_start(
                    dlptp[:, 0:TPb * 128],
                    DLPT[:, blk["ppbase"] * 128:(blk["ppbase"] + TPb) * 128]
                    .to_broadcast([128, TPb * 128]))
                bt = _emit_block_common(nc, pools, blk, i1g, dlpg, dlptp,
                                        iot, ioc, gather_fn, 42)
                T = blk["T"]
                pp_of = {}
                pp = 0
                for (di, pt_lo, pt_hi, lo, hi) in blk["passes"]:
                    for pt in range(pt_lo, pt_hi):
                        pp_of[(di, pt)] = pp
                        pp += 1
                bt["pp_of"] = pp_of

                adc = sb3.tile([128, TMAX], F16, tag="adc2", name="adc2")
                if clevel >= 1:
                    psAD = psadp.tile([128, TMAX], F32, tag="psAD2",
                                      name="psAD2")
                    plist = [(di, pt) for (di, pt_lo, pt_hi, lo, hi)
                             in blk["passes"] for pt in range(pt_lo, pt_hi)]
                    cover_count = {}
                    for di, pt in plist:
                        cover_count[pt] = cover_count.get(pt, 0) + 1
                    seen = {}
                    for di, pt in plist:
                        seen[pt] = seen.get(pt, 0) + 1
                        pp_off = pp_of[(di, pt)]
                        nc.tensor.matmul(
                            psAD[:, pt:pt + 1],
                            lhsT=bt["indT"][:, pp_off * 128:(pp_off + 1) * 128],
                            rhs=ad2g[:, di:di + 1],
                            start=(seen[pt] == 1),
                            stop=(seen[pt] == cover_count[pt]))
                    nc.vector.tensor_copy(adc[:, 0:T], psAD[:, 0:T])
                else:
                    nc.vector.memset(adc[:, 0:T], 0.0)

                g23 = bt["g"][:, 0:T * 42].rearrange("p (t e) -> p t e", e=42)
                at2 = sb3.tile([128, TMAX], F16, tag="at2", name="at2")
                at23 = at2[:, 0:T].rearrange("p (t h) -> p t h", h=1)
                nc.vector.tensor_tensor(
                    out=at23, in0=g23[:, :, 40:41],
                    in1=adc[:, 0:T].rearrange("p (t h) -> p t h", h=1),
                    op=aadd)
                nc.vector.scalar_tensor_tensor(
                    out=at23, in0=at23, scalar=NEG_SLOPE, in1=at23,
                    op0=mult, op1=amax)
                w2t = sb3.tile([128, TMAX], F16, tag="w2t", name="w2t")
                nc.scalar.activation(out=w2t[:, 0:T], in_=at2[:, 0:T],
                                     func=AF.Exp)
                gw = sb3.tile([128, TMAX * 42], F16, tag="gw", name="gw")
                nc.vector.tensor_tensor(
                    out=gw[:, 0:T * 42].rearrange("p (t e) -> p t e", e=42),
                    in0=g23,
                    in1=w2t[:, 0:T].rearrange("p (t s) -> p t s", s=1)
                    .to_broadcast([128, T, 42]),
                    op=mult)
                bt["gw"] = gw
                pending.append((s, bt))
                if len(pending) >= 2:
                    emit_scatter(len(pending) - 2)
            emit_scatter(len(pending) - 1)

            # ---- group post: normalize + log_softmax + write ----
            o2g = sb2.tile([128, G * 42], F32, tag="o2g", name="o2g")
            for di in range(nd):
                nc.vector.tensor_copy(o2g[:, di * 42:(di + 1) * 42],
                                      ps2s[di][:])
            o3 = o2g[:, 0:nd * 42].rearrange("p (t e) -> p t e", e=42)
            rc2 = sb2.tile([128, G], F32, tag="rc2", name="rc2")
            nc.vector.reciprocal(
                rc2[:, 0:nd].rearrange("p (t h) -> p t h", h=1),
                o3[:, :, 41:42])
            lg = sb2.tile([128, G * NCLASS], F32, tag="lg", name="lg")
            lg3 = lg[:, 0:nd * NCLASS].rearrange("p (t e) -> p t e", e=NCLASS)
            nc.vector.tensor_tensor(
                out=lg3, in0=o3[:, :, 0:NCLASS],
                in1=rc2[:, 0:nd].rearrange("p (t h) -> p t h", h=1)
                .to_broadcast([128, nd, NCLASS]),
                op=mult)
            nc.vector.tensor_tensor(
                out=lg3, in0=lg3,
                in1=b2t[:].rearrange("p (t e) -> p t e", t=1)
                .to_broadcast([128, nd, NCLASS]),
                op=aadd)
            ex = sb2.tile([128, NCLASS], F32, tag="ex", name="ex")
            smg = sb2.tile([128, G], F32, tag="smg", name="smg")
            for di in range(nd):
                nc.scalar.activation(
                    out=ex[:], in_=lg[:, di * NCLASS:(di + 1) * NCLASS],
                    func=AF.Exp, accum_out=smg[:, di:di + 1])
            lsg = sb2.tile([128, G], F32, tag="lsg", name="lsg")
            nc.scalar.activation(out=lsg[:, 0:nd], in_=smg[:, 0:nd],
                                 func=AF.Ln)
            fin = sb2.tile([128, G * NCLASS], F32, tag="fin", name="fin")
            nc.vector.tensor_tensor(
                out=fin[:, 0:nd * NCLASS]
                .rearrange("p (t e) -> p t e", e=NCLASS),
                in0=lg3,
                in1=lsg[:, 0:nd].rearrange("p (t h) -> p t h", h=1)
                .to_broadcast([128, nd, NCLASS]),
                op=sub)
            r0 = ds[0] * 128
            r1 = (ds[-1] + 1) * 128
            nc.sync.dma_start(
                OUT[r0:r1, :].rearrange("(t p) e -> p t e", p=128),
                fin[:, 0:nd * NCLASS].rearrange("p (t e) -> p t e", e=NCLASS))


_CACHE = {}


def kernel(x, edge_index, W1, att_src1, att_dst1, b1, W2, att_src2, att_dst2, b2):
    x = np.asarray(x, dtype=np.float32)
    edge_index = np.asarray(edge_index)
    in_maps, layout = _prep(np.asarray(x), edge_index,
                            np.asarray(W1), np.asarray(att_src1),
                            np.asarray(att_dst1), np.asarray(W2),
                            np.asarray(att_src2), np.asarray(att_dst2),
                            b1=b1, b2=b2)
    key = layout.key()
    if key not in _CACHE:
        _CACHE[key] = _build(layout)
    nc = _CACHE[key]
    res = run_bass_kernel_spmd(nc, in_maps, core_ids=list(range(NCORES)))
    out = np.concatenate([res.results[k]["out"][:NPC] for k in range(NCORES)],
                         axis=0)
    return out.astype(np.float32)
